# revision 6
# baseline (speedup 1.0000x reference)
"""Chamfer distance loss on 8 Trainium2 NeuronCores.

Data-parallel over the batch (B=8): core b computes the chamfer loss of
sample b against its own 4096x4096 distance matrix; the host averages the
8 per-sample scalars.

Per-core algorithm (N = M = 4096, D = 3):
  d2[n, m] = |fg_n|^2 + |prj_m|^2 - 2 fg_n . prj_m  is computed on the
  TensorEngine as a single K=5 matmul of host-augmented operands
    fg_aug  = [fg2, 1, -2 fg_x, -2 fg_y, -2 fg_z]   (5 x N, stationary)
    prj_aug = [1, prj2, prj_x, prj_y, prj_z]        (5 x M, moving)
  in [128 n x 2048 m] PSUM groups (4 banks).  One DVE tensor_tensor_reduce
  per group drains PSUM to SBUF bf16 (relu-fused via max(d2, 0)) while
  min-reducing over m into a per-(n-tile, m-group) rowmin slot; a second
  DVE tensor_tensor folds the group into a running bf16 colmin buffer
  (elementwise min across n-tiles).  Padded fg rows (sentinel 10000.0)
  produce d2 ~ 3e8, so they never win either min; the rowmin contribution
  of padded rows is zeroed by a host-built (mask/L) multiply.
  Finally colmin [128, 4096] is min-reduced across partitions with 32
  TensorE transposes + one DVE min-reduce, and both chamfer terms are
  summed across partitions with a K=128 matmul against a ones vector.
"""

import sys

sys.path.insert(0, "/opt/trn_rl_repo")
sys.path.insert(0, "/root/.axon_site/_ro/trn_rl_repo")

import numpy as np

import concourse.bass as bass
import concourse.mybir as mybir
import concourse.tile as tile
from concourse.masks import make_identity

B, N, M, D = 8, 4096, 4096, 3
PAD = 10000.0
P = 128  # partitions / n-tile rows
MG = 2048  # m elements per PSUM group (4 banks)
N_TILES = N // P  # 32
N_GROUPS = M // MG  # 2
BIG = 1.0e30

_cached = {}


def _patch_tile_commit_waits():
    """This walrus build rejects >1 sync-wait per instruction: hoist extra
    waits onto nofuse NOPs committed just before the instruction on the same
    engine (engine streams are in-order, so prefix waits are equivalent)."""
    if getattr(tile.TileContext, "_wait_split_patched", False):
        return
    orig_commit = tile.TileContext._commit_instruction

    def _commit_split(self, inst, lazy_reg_writes=True):
        si = getattr(inst, "sync_info", None)
        eng = getattr(inst, "engine", None)
        if (
            si is not None
            and si.on_wait
            and len(si.on_wait) > 1
            and eng is not None
            and eng != mybir.EngineType.Unassigned
        ):
            waits = list(si.on_wait)
            si.on_wait = waits[:1]
            for w in waits[1:]:
                nop = mybir.InstNoOp(
                    name=f"I-{self.nc.next_id()}",
                    sync_info=mybir.SyncInfo(on_wait=[w], on_update=[]),
                    bass_nofuse=True,
                    engine=eng,
                )
                orig_commit(self, nop, lazy_reg_writes=False)
        return orig_commit(self, inst, lazy_reg_writes)

    tile.TileContext._commit_instruction = _commit_split
    tile.TileContext._wait_split_patched = True


def _patch_tile_tail_drain():
    """This walrus build rejects >1 sync-wait on a TPB_CTRL (Drain)
    instruction; split the TileContext tail-drain's wait list across a chain
    of single-wait drains on the sync engine."""
    from bass_rust import ScopedClock

    def _drain_and_barrier(self, tick_clock, wait_clock):
        nc = self.nc
        drain_inst = nc.sync.drain()
        wait_clock.add_sem_waits(
            drain_inst.ins, ScopedClock({None: tick_clock.global_clock})
        )
        si = drain_inst.ins.sync_info
        waits = list(si.on_wait) if si is not None and si.on_wait else []
        if len(waits) > 1:
            si.on_wait = waits[:1]
            for w in waits[1:]:
                extra = nc.sync.drain()
                esi = extra.ins.sync_info
                if esi is None:
                    extra.ins.sync_info = type(si)(on_wait=[w], on_update=[])
                else:
                    esi.on_wait = [w]

        nc.all_engine_barrier()
        assert self.sems is not None
        popped = nc._tile_sem_poison_stack.pop()
        assert popped is self._sem_poison
        nc.clear_and_free_semaphores(list(self.sems.allocated().values()))
        nc.all_engine_barrier()

    tile.TileContext._drain_and_barrier = _drain_and_barrier


def _build_program():
    _patch_tile_commit_waits()
    _patch_tile_tail_drain()
    f32 = mybir.dt.float32
    bf16 = mybir.dt.bfloat16
    Alu = mybir.AluOpType
    Ax = mybir.AxisListType

    nc = bass.Bass("TRN2", target_bir_lowering=False, debug=False, num_devices=B)
    fg_aug = nc.dram_tensor("fg_aug", [5, N], f32, kind="ExternalInput").ap()
    prj_aug = nc.dram_tensor("prj_aug", [5, M], f32, kind="ExternalInput").ap()
    mask = nc.dram_tensor("mask", [P, N_TILES], f32, kind="ExternalInput").ap()
    out = nc.dram_tensor("out", [1, 1], f32, kind="ExternalOutput").ap()

    with tile.TileContext(nc) as tc:
        with (
            tc.tile_pool(name="consts", bufs=1) as consts,
            tc.tile_pool(name="d2p", bufs=3) as d2p,
            tc.tile_pool(name="psum", bufs=2, space="PSUM") as psum,
        ):
            fg_sb = consts.tile([5, N], f32)
            prj_sb = consts.tile([5, M], f32)
            mask_sb = consts.tile([P, N_TILES], f32)
            nc.sync.dma_start(out=fg_sb[:], in_=fg_aug)
            nc.sync.dma_start(out=prj_sb[:], in_=prj_aug)
            nc.sync.dma_start(out=mask_sb[:], in_=mask)

            ones_sb = consts.tile([P, 1], f32)
            nc.vector.memset(ones_sb[:], 1.0)
            ident_sb = consts.tile([P, P], bf16)
            make_identity(nc, ident_sb)

            colmin = consts.tile([P, M], bf16)
            nc.gpsimd.memset(colmin[:], BIG)
            rowmin_parts = consts.tile([P, N_TILES, N_GROUPS], f32)

            # ---- main loop: 32 n-tiles x 2 m-groups ----
            for i in range(N_TILES):
                lhsT = fg_sb[:, i * P : (i + 1) * P]
                for g in range(N_GROUPS):
                    grp = psum.tile([P, MG], f32, tag="grp")
                    for j in range(MG // 512):
                        m0 = g * MG + j * 512
                        nc.tensor.matmul(
                            grp[:, j * 512 : (j + 1) * 512],
                            lhsT,
                            prj_sb[:, m0 : m0 + 512],
                            start=True,
                            stop=True,
                        )
                    d2 = d2p.tile([P, MG], bf16)
                    # drain PSUM -> SBUF bf16 with fused relu on ScalarE
                    nc.scalar.activation(
                        d2[:], grp[:], mybir.ActivationFunctionType.Relu
                    )
                    # rowmin slot = min_m d2
                    nc.vector.tensor_reduce(
                        rowmin_parts[:, i, g : g + 1], d2[:], axis=Ax.X, op=Alu.min
                    )
                    # colmin slice = min(colmin slice, d2)
                    cslice = colmin[:, g * MG : (g + 1) * MG]
                    nc.vector.tensor_tensor(cslice, d2[:], cslice, Alu.min)

            # ---- cham_x: rowmin -> masked mean over valid rows ----
            rowmin2 = consts.tile([P, N_TILES], f32)
            nc.vector.tensor_reduce(
                rowmin2[:], rowmin_parts[:], axis=Ax.X, op=Alu.min
            )
            rowx = consts.tile([P, N_TILES], f32)
            # rowx = max(rowmin2, 0) * (mask/L)
            nc.vector.scalar_tensor_tensor(
                rowx[:], rowmin2[:], 0.0, mask_sb[:], op0=Alu.max, op1=Alu.mult
            )

            # ---- cham_y: colmin across partitions via PE transposes ----
            tp = psum.tile([P, N_TILES, P], bf16, tag="grp")
            for bank in range(4):
                for k in range(8):
                    c = bank * 8 + k
                    nc.tensor.transpose(
                        tp[:, c, :],
                        colmin[:, c * P : (c + 1) * P],
                        ident_sb[:],
                    )
            colmin2 = consts.tile([P, N_TILES], f32)
            nc.vector.tensor_reduce(colmin2[:], tp[:], axis=Ax.X, op=Alu.min)
            coly = consts.tile([P, N_TILES], f32)
            # coly = max(colmin2, 0) * (1/M)
            nc.vector.tensor_scalar(
                out=coly[:],
                in0=colmin2[:],
                scalar1=0.0,
                scalar2=1.0 / M,
                op0=Alu.max,
                op1=Alu.mult,
            )

            # ---- total = sum_p sum_i (rowx + coly) via K=128 matmul ----
            tot32 = consts.tile([P, N_TILES], f32)
            nc.vector.tensor_add(tot32[:], rowx[:], coly[:])
            tot = consts.tile([P, 1], f32)
            nc.vector.tensor_reduce(tot[:], tot32[:], axis=Ax.X, op=Alu.add)
            px = psum.tile([1, 512], f32, tag="grp")
            nc.tensor.matmul(px[:, 0:1], tot[:], ones_sb[:], start=True, stop=True)
            res = consts.tile([1, 1], f32)
            nc.vector.tensor_copy(out=res[:], in_=px[:, 0:1])
            nc.sync.dma_start(out=out, in_=res[:])

    return nc


def _prep_core_inputs(fg, prj, length):
    """Host-side prep for one sample: augmented matmul operands + mask."""
    f = fg.astype(np.float32)
    p = prj.astype(np.float32)
    L = int(length)
    fg2 = (f.astype(np.float64) ** 2).sum(-1).astype(np.float32)
    prj2 = (p.astype(np.float64) ** 2).sum(-1).astype(np.float32)
    fg_aug = np.ascontiguousarray(
        np.stack([fg2, np.ones(N, np.float32), -2 * f[:, 0], -2 * f[:, 1], -2 * f[:, 2]])
    )
    prj_aug = np.ascontiguousarray(
        np.stack([np.ones(M, np.float32), prj2, p[:, 0], p[:, 1], p[:, 2]])
    )
    mask = (np.arange(N) < L).astype(np.float32).reshape(N_TILES, P).T / L
    return {
        "fg_aug": fg_aug,
        "prj_aug": prj_aug,
        "mask": np.ascontiguousarray(mask),
    }


def _run(in_maps, trace=False):
    from concourse.bass_utils import run_bass_kernel_spmd

    if "nc" not in _cached:
        _cached["nc"] = _build_program()
    return run_bass_kernel_spmd(
        _cached["nc"], in_maps, list(range(B)), trace=trace
    )


def kernel(fg_points, prj_points, x_lengths, _trace=False):
    fg = np.asarray(fg_points)
    prj = np.asarray(prj_points)
    lengths = np.asarray(x_lengths)
    in_maps = [
        _prep_core_inputs(fg[b], prj[b], lengths[b]) for b in range(B)
    ]
    res = _run(in_maps, trace=_trace)
    vals = [float(res.results[b]["out"][0, 0]) for b in range(B)]
    out = np.array(np.mean(vals), dtype=np.float32)
    if _trace:
        return out, res
    return out


# revision 10
# speedup vs baseline: 1.8857x; 1.8857x over previous
"""Chamfer distance loss on 8 Trainium2 NeuronCores.

Data-parallel over the batch (B=8): core b computes the chamfer loss of
sample b against its own 4096x4096 distance matrix; the host averages the
8 per-sample scalars.

Per-core algorithm (N = M = 4096, D = 3):
  d2[n, m] = |fg_n|^2 + |prj_m|^2 - 2 fg_n . prj_m  is computed on the
  TensorEngine as a single K=5 matmul of host-augmented operands
    fg_aug  = [fg2, 1, -2 fg_x, -2 fg_y, -2 fg_z]   (5 x N, stationary)
    prj_aug = [1, prj2, prj_x, prj_y, prj_z]        (5 x M, moving)
  in [128 n x 2048 m] PSUM groups (4 banks).  One DVE tensor_tensor_reduce
  per group drains PSUM to SBUF bf16 (relu-fused via max(d2, 0)) while
  min-reducing over m into a per-(n-tile, m-group) rowmin slot; a second
  DVE tensor_tensor folds the group into a running bf16 colmin buffer
  (elementwise min across n-tiles).  Padded fg rows (sentinel 10000.0)
  produce d2 ~ 3e8, so they never win either min; the rowmin contribution
  of padded rows is zeroed by a host-built (mask/L) multiply.
  Finally colmin [128, 4096] is min-reduced across partitions with 32
  TensorE transposes + one DVE min-reduce, and both chamfer terms are
  summed across partitions with a K=128 matmul against a ones vector.
"""

import sys

sys.path.insert(0, "/opt/trn_rl_repo")
sys.path.insert(0, "/root/.axon_site/_ro/trn_rl_repo")

import numpy as np

import concourse.bass as bass
import concourse.mybir as mybir
import concourse.tile as tile
from concourse.masks import make_identity

B, N, M, D = 8, 4096, 4096, 3
PAD = 10000.0
P = 128  # partitions / n-tile rows
MG = 2048  # m elements per PSUM group (4 banks)
N_TILES = N // P  # 32
N_GROUPS = M // MG  # 2
BIG = 1.0e30

_cached = {}


def _patch_tile_commit_waits():
    """This walrus build rejects >1 sync-wait per instruction: hoist extra
    waits onto nofuse NOPs committed just before the instruction on the same
    engine (engine streams are in-order, so prefix waits are equivalent)."""
    if getattr(tile.TileContext, "_wait_split_patched", False):
        return
    orig_commit = tile.TileContext._commit_instruction

    def _commit_split(self, inst, lazy_reg_writes=True):
        si = getattr(inst, "sync_info", None)
        eng = getattr(inst, "engine", None)
        if (
            si is not None
            and si.on_wait
            and len(si.on_wait) > 1
            and eng is not None
            and eng != mybir.EngineType.Unassigned
        ):
            waits = list(si.on_wait)
            si.on_wait = waits[:1]
            for w in waits[1:]:
                nop = mybir.InstNoOp(
                    name=f"I-{self.nc.next_id()}",
                    sync_info=mybir.SyncInfo(on_wait=[w], on_update=[]),
                    bass_nofuse=True,
                    engine=eng,
                )
                orig_commit(self, nop, lazy_reg_writes=False)
        return orig_commit(self, inst, lazy_reg_writes)

    tile.TileContext._commit_instruction = _commit_split
    tile.TileContext._wait_split_patched = True


def _patch_tile_tail_drain():
    """This walrus build rejects >1 sync-wait on a TPB_CTRL (Drain)
    instruction; split the TileContext tail-drain's wait list across a chain
    of single-wait drains on the sync engine."""
    from bass_rust import ScopedClock

    def _drain_and_barrier(self, tick_clock, wait_clock):
        nc = self.nc
        drain_inst = nc.sync.drain()
        wait_clock.add_sem_waits(
            drain_inst.ins, ScopedClock({None: tick_clock.global_clock})
        )
        si = drain_inst.ins.sync_info
        waits = list(si.on_wait) if si is not None and si.on_wait else []
        if len(waits) > 1:
            si.on_wait = waits[:1]
            for w in waits[1:]:
                extra = nc.sync.drain()
                esi = extra.ins.sync_info
                if esi is None:
                    extra.ins.sync_info = type(si)(on_wait=[w], on_update=[])
                else:
                    esi.on_wait = [w]

        nc.all_engine_barrier()
        assert self.sems is not None
        popped = nc._tile_sem_poison_stack.pop()
        assert popped is self._sem_poison
        nc.clear_and_free_semaphores(list(self.sems.allocated().values()))
        nc.all_engine_barrier()

    tile.TileContext._drain_and_barrier = _drain_and_barrier


def _build_program():
    _patch_tile_commit_waits()
    _patch_tile_tail_drain()
    f32 = mybir.dt.float32
    bf16 = mybir.dt.bfloat16
    Alu = mybir.AluOpType
    Ax = mybir.AxisListType

    nc = bass.Bass("TRN2", target_bir_lowering=False, debug=False, num_devices=B)
    fg_aug = nc.dram_tensor("fg_aug", [10, N], bf16, kind="ExternalInput").ap()
    prj_a = nc.dram_tensor("prj_a", [10, M], bf16, kind="ExternalInput").ap()
    prj_b = nc.dram_tensor("prj_b", [6, M], bf16, kind="ExternalInput").ap()
    fg_b = nc.dram_tensor("fg_b", [6, N], bf16, kind="ExternalInput").ap()
    mask = nc.dram_tensor("mask", [P, N_TILES], f32, kind="ExternalInput").ap()
    out = nc.dram_tensor("out", [1, 1], f32, kind="ExternalOutput").ap()

    with tile.TileContext(nc) as tc:
        with (
            tc.tile_pool(name="consts", bufs=1) as consts,
            tc.tile_pool(name="d2p", bufs=3) as d2p,
            tc.tile_pool(name="foldp", bufs=3) as foldp,
            tc.tile_pool(name="psum", bufs=2, space="PSUM") as psum,
        ):
            fg_sb = consts.tile([10, N], bf16)
            prja_sb = consts.tile([10, M], bf16)
            prjb_sb = consts.tile([6, M], bf16)
            fgb_sb = consts.tile([6, N], bf16)
            mask_sb = consts.tile([P, N_TILES], f32)
            nc.sync.dma_start(out=fg_sb[:], in_=fg_aug)
            nc.sync.dma_start(out=prja_sb[:], in_=prj_a)
            nc.sync.dma_start(out=prjb_sb[:], in_=prj_b)
            nc.sync.dma_start(out=fgb_sb[:], in_=fg_b)
            nc.sync.dma_start(out=mask_sb[:], in_=mask)

            ones_sb = consts.tile([P, 1], f32)
            nc.vector.memset(ones_sb[:], 1.0)
            ident_sb = consts.tile([P, P], bf16)
            make_identity(nc, ident_sb)

            colmin = consts.tile([P, M], bf16)
            nc.gpsimd.memset(colmin[:], BIG)
            rowmin_parts = consts.tile([P, N_TILES, N_GROUPS], f32)

            # ---- main loop: 32 n-tiles x 2 m-groups ----
            for i in range(N_TILES):
                lhsT = fg_sb[:, i * P : (i + 1) * P]
                lhsT2 = fgb_sb[:, i * P : (i + 1) * P]
                for g in range(N_GROUPS):
                    grp = psum.tile([P, MG], f32, tag="grp")
                    for j in range(MG // 512):
                        m0 = g * MG + j * 512
                        # d2 = fg2 + prj2 - 2 fg.prj at ~f32 precision via
                        # round-to-nearest bf16 hi/lo splits: MM1 covers all
                        # terms against p_hi, MM2 accumulates -2 fg . p_lo.
                        nc.tensor.matmul(
                            grp[:, j * 512 : (j + 1) * 512],
                            lhsT,
                            prja_sb[:, m0 : m0 + 512],
                            start=True,
                            stop=False,
                        )
                        nc.tensor.matmul(
                            grp[:, j * 512 : (j + 1) * 512],
                            lhsT2,
                            prjb_sb[:, m0 : m0 + 512],
                            start=False,
                            stop=True,
                        )
                    d2 = d2p.tile([P, MG], bf16)
                    # drain PSUM -> SBUF bf16 with fused relu on ScalarE
                    nc.scalar.activation(
                        d2[:], grp[:], mybir.ActivationFunctionType.Relu
                    )
                    # colmin slice = min(colmin slice, d2)
                    cslice = colmin[:, g * MG : (g + 1) * MG]
                    nc.vector.tensor_tensor(cslice, d2[:], cslice, Alu.min)
                    # rowmin: two 2x-mode TT-min folds, then a small 1x reduce
                    h1 = foldp.tile([P, MG // 2], bf16)
                    nc.vector.tensor_tensor(
                        h1[:], d2[:, : MG // 2], d2[:, MG // 2 :], Alu.min
                    )
                    nc.vector.tensor_tensor(
                        h1[:, : MG // 4], h1[:, : MG // 4], h1[:, MG // 4 :], Alu.min
                    )
                    nc.vector.tensor_reduce(
                        rowmin_parts[:, i, g : g + 1],
                        h1[:, : MG // 4],
                        axis=Ax.X,
                        op=Alu.min,
                    )

            # ---- cham_x: rowmin -> masked mean over valid rows ----
            rowmin2 = consts.tile([P, N_TILES], f32)
            nc.vector.tensor_reduce(
                rowmin2[:], rowmin_parts[:], axis=Ax.X, op=Alu.min
            )
            rowx = consts.tile([P, N_TILES], f32)
            # rowx = max(rowmin2, 0) * (mask/L)
            nc.vector.scalar_tensor_tensor(
                rowx[:], rowmin2[:], 0.0, mask_sb[:], op0=Alu.max, op1=Alu.mult
            )

            # ---- cham_y: colmin across partitions via PE transposes ----
            tp = psum.tile([P, N_TILES, P], bf16, tag="grp")
            for bank in range(4):
                for k in range(8):
                    c = bank * 8 + k
                    nc.tensor.transpose(
                        tp[:, c, :],
                        colmin[:, c * P : (c + 1) * P],
                        ident_sb[:],
                    )
            colmin2 = consts.tile([P, N_TILES], f32)
            nc.vector.tensor_reduce(colmin2[:], tp[:], axis=Ax.X, op=Alu.min)
            coly = consts.tile([P, N_TILES], f32)
            # coly = max(colmin2, 0) * (1/M)
            nc.vector.tensor_scalar(
                out=coly[:],
                in0=colmin2[:],
                scalar1=0.0,
                scalar2=1.0 / M,
                op0=Alu.max,
                op1=Alu.mult,
            )

            # ---- total = sum_p sum_i (rowx + coly) via K=128 matmul ----
            tot32 = consts.tile([P, N_TILES], f32)
            nc.vector.tensor_add(tot32[:], rowx[:], coly[:])
            tot = consts.tile([P, 1], f32)
            nc.vector.tensor_reduce(tot[:], tot32[:], axis=Ax.X, op=Alu.add)
            px = psum.tile([1, 512], f32, tag="grp")
            nc.tensor.matmul(px[:, 0:1], tot[:], ones_sb[:], start=True, stop=True)
            res = consts.tile([1, 1], f32)
            nc.vector.tensor_copy(out=res[:], in_=px[:, 0:1])
            nc.sync.dma_start(out=out, in_=res[:])

    return nc


def _split_bf16(x):
    """Round-to-nearest bf16 hi/lo split: x ~= hi + lo to ~16 mantissa bits."""
    import ml_dtypes

    hi = x.astype(np.float32).astype(ml_dtypes.bfloat16)
    lo = (x.astype(np.float32) - hi.astype(np.float32)).astype(ml_dtypes.bfloat16)
    return hi, lo


def _prep_core_inputs(fg, prj, length):
    """Host-side prep for one sample: hi/lo-split augmented matmul operands.

    MM1 (K=10): lhsT=[a_hi,a_lo,1,1,-2fh,-2fl] rhs=[1,1,b_hi,b_lo,ph,ph]
      -> fg2 + prj2 - 2 fg . p_hi
    MM2 (K=6):  lhsT=[-2fh,-2fl]               rhs=[pl,pl]
      -> -2 fg . p_lo   (PSUM-accumulated onto MM1)
    """
    import ml_dtypes

    bf = ml_dtypes.bfloat16
    f = fg.astype(np.float32)
    p = prj.astype(np.float32)
    L = int(length)
    fg2 = (f.astype(np.float64) ** 2).sum(-1).astype(np.float32)
    prj2 = (p.astype(np.float64) ** 2).sum(-1).astype(np.float32)
    a_hi, a_lo = _split_bf16(fg2)
    b_hi, b_lo = _split_bf16(prj2)
    f_hi, f_lo = _split_bf16(f)  # [N, 3] each
    p_hi, p_lo = _split_bf16(p)  # [M, 3] each
    ones_n = np.ones(N, bf)
    ones_m = np.ones(M, bf)
    f2_hi = (-2.0 * f_hi.astype(np.float32)).astype(bf)  # exact scale by -2
    f2_lo = (-2.0 * f_lo.astype(np.float32)).astype(bf)
    fg_aug = np.ascontiguousarray(
        np.stack(
            [a_hi, a_lo, ones_n, ones_n]
            + [f2_hi[:, d] for d in range(3)]
            + [f2_lo[:, d] for d in range(3)]
        )
    )
    prj_a = np.ascontiguousarray(
        np.stack(
            [ones_m, ones_m, b_hi, b_lo]
            + [p_hi[:, d] for d in range(3)]
            + [p_hi[:, d] for d in range(3)]
        )
    )
    prj_b = np.ascontiguousarray(
        np.stack([p_lo[:, d] for d in range(3)] + [p_lo[:, d] for d in range(3)])
    )
    fg_b = np.ascontiguousarray(
        np.stack([f2_hi[:, d] for d in range(3)] + [f2_lo[:, d] for d in range(3)])
    )
    mask = (np.arange(N) < L).astype(np.float32).reshape(N_TILES, P).T / L
    return {
        "fg_aug": fg_aug,
        "prj_a": prj_a,
        "prj_b": prj_b,
        "fg_b": fg_b,
        "mask": np.ascontiguousarray(mask),
    }


def _run(in_maps, trace=False):
    from concourse.bass_utils import run_bass_kernel_spmd

    if "nc" not in _cached:
        _cached["nc"] = _build_program()
    return run_bass_kernel_spmd(
        _cached["nc"], in_maps, list(range(B)), trace=trace
    )


def kernel(fg_points, prj_points, x_lengths, _trace=False):
    fg = np.asarray(fg_points)
    prj = np.asarray(prj_points)
    lengths = np.asarray(x_lengths)
    in_maps = [
        _prep_core_inputs(fg[b], prj[b], lengths[b]) for b in range(B)
    ]
    res = _run(in_maps, trace=_trace)
    vals = [float(res.results[b]["out"][0, 0]) for b in range(B)]
    out = np.array(np.mean(vals), dtype=np.float32)
    if _trace:
        return out, res
    return out


# revision 12
# speedup vs baseline: 2.3765x; 1.2603x over previous
"""Chamfer distance loss on 8 Trainium2 NeuronCores.

Data-parallel over the batch (B=8): core b computes the chamfer loss of
sample b against its own 4096x4096 distance matrix; the host averages the
8 per-sample scalars.

Per-core algorithm (N = M = 4096, D = 3):
  d2[n, m] = |fg_n|^2 + |prj_m|^2 - 2 fg_n . prj_m  is computed on the
  TensorEngine as a single K=5 matmul of host-augmented operands
    fg_aug  = [fg2, 1, -2 fg_x, -2 fg_y, -2 fg_z]   (5 x N, stationary)
    prj_aug = [1, prj2, prj_x, prj_y, prj_z]        (5 x M, moving)
  in [128 n x 2048 m] PSUM groups (4 banks).  One DVE tensor_tensor_reduce
  per group drains PSUM to SBUF bf16 (relu-fused via max(d2, 0)) while
  min-reducing over m into a per-(n-tile, m-group) rowmin slot; a second
  DVE tensor_tensor folds the group into a running bf16 colmin buffer
  (elementwise min across n-tiles).  Padded fg rows (sentinel 10000.0)
  produce d2 ~ 3e8, so they never win either min; the rowmin contribution
  of padded rows is zeroed by a host-built (mask/L) multiply.
  Finally colmin [128, 4096] is min-reduced across partitions with 32
  TensorE transposes + one DVE min-reduce, and both chamfer terms are
  summed across partitions with a K=128 matmul against a ones vector.
"""

import sys

sys.path.insert(0, "/opt/trn_rl_repo")
sys.path.insert(0, "/root/.axon_site/_ro/trn_rl_repo")

import numpy as np

import concourse.bass as bass
import concourse.mybir as mybir
import concourse.tile as tile
from concourse.masks import make_identity

B, N, M, D = 8, 4096, 4096, 3
PAD = 10000.0
P = 128  # partitions / n-tile rows
MG = 2048  # m elements per PSUM group (4 banks)
N_TILES = N // P  # 32
N_GROUPS = M // MG  # 2
BIG = 1.0e30

_cached = {}


def _patch_tile_commit_waits():
    """This walrus build rejects >1 sync-wait per instruction: hoist extra
    waits onto nofuse NOPs committed just before the instruction on the same
    engine (engine streams are in-order, so prefix waits are equivalent)."""
    if getattr(tile.TileContext, "_wait_split_patched", False):
        return
    orig_commit = tile.TileContext._commit_instruction

    def _commit_split(self, inst, lazy_reg_writes=True):
        si = getattr(inst, "sync_info", None)
        eng = getattr(inst, "engine", None)
        if (
            si is not None
            and si.on_wait
            and len(si.on_wait) > 1
            and eng is not None
            and eng != mybir.EngineType.Unassigned
        ):
            waits = list(si.on_wait)
            si.on_wait = waits[:1]
            for w in waits[1:]:
                nop = mybir.InstNoOp(
                    name=f"I-{self.nc.next_id()}",
                    sync_info=mybir.SyncInfo(on_wait=[w], on_update=[]),
                    bass_nofuse=True,
                    engine=eng,
                )
                orig_commit(self, nop, lazy_reg_writes=False)
        return orig_commit(self, inst, lazy_reg_writes)

    tile.TileContext._commit_instruction = _commit_split
    tile.TileContext._wait_split_patched = True


def _patch_tile_tail_drain():
    """This walrus build rejects >1 sync-wait on a TPB_CTRL (Drain)
    instruction; split the TileContext tail-drain's wait list across a chain
    of single-wait drains on the sync engine."""
    from bass_rust import ScopedClock

    def _drain_and_barrier(self, tick_clock, wait_clock):
        nc = self.nc
        drain_inst = nc.sync.drain()
        wait_clock.add_sem_waits(
            drain_inst.ins, ScopedClock({None: tick_clock.global_clock})
        )
        si = drain_inst.ins.sync_info
        waits = list(si.on_wait) if si is not None and si.on_wait else []
        if len(waits) > 1:
            si.on_wait = waits[:1]
            for w in waits[1:]:
                extra = nc.sync.drain()
                esi = extra.ins.sync_info
                if esi is None:
                    extra.ins.sync_info = type(si)(on_wait=[w], on_update=[])
                else:
                    esi.on_wait = [w]

        nc.all_engine_barrier()
        assert self.sems is not None
        popped = nc._tile_sem_poison_stack.pop()
        assert popped is self._sem_poison
        nc.clear_and_free_semaphores(list(self.sems.allocated().values()))
        nc.all_engine_barrier()

    tile.TileContext._drain_and_barrier = _drain_and_barrier


def _build_program():
    _patch_tile_commit_waits()
    _patch_tile_tail_drain()
    f32 = mybir.dt.float32
    bf16 = mybir.dt.bfloat16
    Alu = mybir.AluOpType
    Ax = mybir.AxisListType

    nc = bass.Bass("TRN2", target_bir_lowering=False, debug=False, num_devices=B)
    fg_aug = nc.dram_tensor("fg_aug", [16, N], bf16, kind="ExternalInput").ap()
    prj_aug = nc.dram_tensor("prj_aug", [16, M], bf16, kind="ExternalInput").ap()
    mask = nc.dram_tensor("mask", [P, N_TILES], f32, kind="ExternalInput").ap()
    out = nc.dram_tensor("out", [1, 1], f32, kind="ExternalOutput").ap()

    with tile.TileContext(nc) as tc:
        with (
            tc.tile_pool(name="consts", bufs=1) as consts,
            tc.tile_pool(name="d2p", bufs=3) as d2p,
            tc.tile_pool(name="foldp", bufs=3) as foldp,
            tc.tile_pool(name="psum", bufs=2, space="PSUM") as psum,
        ):
            fg_sb = consts.tile([16, N], bf16)
            prj_sb = consts.tile([16, M], bf16)
            mask_sb = consts.tile([P, N_TILES], f32)
            nc.sync.dma_start(out=fg_sb[:], in_=fg_aug)
            nc.sync.dma_start(out=prj_sb[:], in_=prj_aug)
            nc.sync.dma_start(out=mask_sb[:], in_=mask)

            ones_sb = consts.tile([P, 1], f32)
            nc.vector.memset(ones_sb[:], 1.0)
            ident_sb = consts.tile([P, P], bf16)
            make_identity(nc, ident_sb)

            colmin = consts.tile([P, M], bf16)
            nc.gpsimd.memset(colmin[:], BIG)
            rowmin_parts = consts.tile([P, N_TILES, N_GROUPS], f32)

            # ---- main loop: 32 n-tiles x 2 m-groups ----
            for i in range(N_TILES):
                lhsT = fg_sb[:, i * P : (i + 1) * P]
                for g in range(N_GROUPS):
                    grp = psum.tile([P, MG], f32, tag="grp")
                    for j in range(MG // 512):
                        m0 = g * MG + j * 512
                        # d2 = fg2 + prj2 - 2 fg.prj at ~f32 precision: one
                        # K=16 bf16 matmul over round-to-nearest hi/lo splits
                        # (all hi- and lo-cross-terms in the contraction).
                        nc.tensor.matmul(
                            grp[:, j * 512 : (j + 1) * 512],
                            lhsT,
                            prj_sb[:, m0 : m0 + 512],
                            start=True,
                            stop=True,
                        )
                    d2 = d2p.tile([P, MG], bf16)
                    # drain PSUM -> SBUF bf16 with fused relu on ScalarE
                    nc.scalar.activation(
                        d2[:], grp[:], mybir.ActivationFunctionType.Relu
                    )
                    # colmin slice = min(colmin slice, d2)
                    cslice = colmin[:, g * MG : (g + 1) * MG]
                    nc.vector.tensor_tensor(cslice, d2[:], cslice, Alu.min)
                    # rowmin: two 2x-mode TT-min folds, then a small 1x reduce
                    h1 = foldp.tile([P, MG // 2], bf16)
                    nc.vector.tensor_tensor(
                        h1[:], d2[:, : MG // 2], d2[:, MG // 2 :], Alu.min
                    )
                    nc.vector.tensor_tensor(
                        h1[:, : MG // 4], h1[:, : MG // 4], h1[:, MG // 4 :], Alu.min
                    )
                    nc.vector.tensor_reduce(
                        rowmin_parts[:, i, g : g + 1],
                        h1[:, : MG // 4],
                        axis=Ax.X,
                        op=Alu.min,
                    )

            # ---- cham_x: rowmin -> masked mean over valid rows ----
            rowmin2 = consts.tile([P, N_TILES], f32)
            nc.vector.tensor_reduce(
                rowmin2[:], rowmin_parts[:], axis=Ax.X, op=Alu.min
            )
            rowx = consts.tile([P, N_TILES], f32)
            # rowx = max(rowmin2, 0) * (mask/L)
            nc.vector.scalar_tensor_tensor(
                rowx[:], rowmin2[:], 0.0, mask_sb[:], op0=Alu.max, op1=Alu.mult
            )

            # ---- cham_y: colmin across partitions via PE transposes ----
            tp = psum.tile([P, N_TILES, P], bf16, tag="grp")
            for bank in range(4):
                for k in range(8):
                    c = bank * 8 + k
                    nc.tensor.transpose(
                        tp[:, c, :],
                        colmin[:, c * P : (c + 1) * P],
                        ident_sb[:],
                    )
            colmin2 = consts.tile([P, N_TILES], f32)
            nc.vector.tensor_reduce(colmin2[:], tp[:], axis=Ax.X, op=Alu.min)
            coly = consts.tile([P, N_TILES], f32)
            # coly = max(colmin2, 0) * (1/M)
            nc.vector.tensor_scalar(
                out=coly[:],
                in0=colmin2[:],
                scalar1=0.0,
                scalar2=1.0 / M,
                op0=Alu.max,
                op1=Alu.mult,
            )

            # ---- total = sum_p sum_i (rowx + coly) via K=128 matmul ----
            tot32 = consts.tile([P, N_TILES], f32)
            nc.vector.tensor_add(tot32[:], rowx[:], coly[:])
            tot = consts.tile([P, 1], f32)
            nc.vector.tensor_reduce(tot[:], tot32[:], axis=Ax.X, op=Alu.add)
            px = psum.tile([1, 512], f32, tag="grp")
            nc.tensor.matmul(px[:, 0:1], tot[:], ones_sb[:], start=True, stop=True)
            res = consts.tile([1, 1], f32)
            nc.vector.tensor_copy(out=res[:], in_=px[:, 0:1])
            nc.sync.dma_start(out=out, in_=res[:])

    return nc


def _split_bf16(x):
    """Round-to-nearest bf16 hi/lo split: x ~= hi + lo to ~16 mantissa bits."""
    import ml_dtypes

    hi = x.astype(np.float32).astype(ml_dtypes.bfloat16)
    lo = (x.astype(np.float32) - hi.astype(np.float32)).astype(ml_dtypes.bfloat16)
    return hi, lo


def _prep_core_inputs(fg, prj, length):
    """Host-side prep for one sample: hi/lo-split augmented matmul operands.

    MM1 (K=10): lhsT=[a_hi,a_lo,1,1,-2fh,-2fl] rhs=[1,1,b_hi,b_lo,ph,ph]
      -> fg2 + prj2 - 2 fg . p_hi
    MM2 (K=6):  lhsT=[-2fh,-2fl]               rhs=[pl,pl]
      -> -2 fg . p_lo   (PSUM-accumulated onto MM1)
    """
    import ml_dtypes

    bf = ml_dtypes.bfloat16
    f = fg.astype(np.float32)
    p = prj.astype(np.float32)
    L = int(length)
    fg2 = (f.astype(np.float64) ** 2).sum(-1).astype(np.float32)
    prj2 = (p.astype(np.float64) ** 2).sum(-1).astype(np.float32)
    a_hi, a_lo = _split_bf16(fg2)
    b_hi, b_lo = _split_bf16(prj2)
    f_hi, f_lo = _split_bf16(f)  # [N, 3] each
    p_hi, p_lo = _split_bf16(p)  # [M, 3] each
    ones_n = np.ones(N, bf)
    ones_m = np.ones(M, bf)
    f2_hi = (-2.0 * f_hi.astype(np.float32)).astype(bf)  # exact scale by -2
    f2_lo = (-2.0 * f_lo.astype(np.float32)).astype(bf)
    fg_aug = np.ascontiguousarray(
        np.stack(
            [a_hi, a_lo, ones_n, ones_n]
            + [f2_hi[:, d] for d in range(3)]
            + [f2_lo[:, d] for d in range(3)]
            + [f2_hi[:, d] for d in range(3)]
            + [f2_lo[:, d] for d in range(3)]
        )
    )
    prj_aug = np.ascontiguousarray(
        np.stack(
            [ones_m, ones_m, b_hi, b_lo]
            + [p_hi[:, d] for d in range(3)]
            + [p_hi[:, d] for d in range(3)]
            + [p_lo[:, d] for d in range(3)]
            + [p_lo[:, d] for d in range(3)]
        )
    )
    mask = (np.arange(N) < L).astype(np.float32).reshape(N_TILES, P).T / L
    return {
        "fg_aug": fg_aug,
        "prj_aug": prj_aug,
        "mask": np.ascontiguousarray(mask),
    }


def _run(in_maps, trace=False):
    from concourse.bass_utils import run_bass_kernel_spmd

    if "nc" not in _cached:
        _cached["nc"] = _build_program()
    return run_bass_kernel_spmd(
        _cached["nc"], in_maps, list(range(B)), trace=trace
    )


def kernel(fg_points, prj_points, x_lengths, _trace=False):
    fg = np.asarray(fg_points)
    prj = np.asarray(prj_points)
    lengths = np.asarray(x_lengths)
    in_maps = [
        _prep_core_inputs(fg[b], prj[b], lengths[b]) for b in range(B)
    ]
    res = _run(in_maps, trace=_trace)
    vals = [float(res.results[b]["out"][0, 0]) for b in range(B)]
    out = np.array(np.mean(vals), dtype=np.float32)
    if _trace:
        return out, res
    return out


# revision 13
# speedup vs baseline: 2.4355x; 1.0248x over previous
"""Chamfer distance loss on 8 Trainium2 NeuronCores.

Data-parallel over the batch (B=8): core b computes the chamfer loss of
sample b against its own 4096x4096 distance matrix; the host averages the
8 per-sample scalars.

Per-core algorithm (N = M = 4096, D = 3):
  d2[n, m] = |fg_n|^2 + |prj_m|^2 - 2 fg_n . prj_m  is computed on the
  TensorEngine as a single K=5 matmul of host-augmented operands
    fg_aug  = [fg2, 1, -2 fg_x, -2 fg_y, -2 fg_z]   (5 x N, stationary)
    prj_aug = [1, prj2, prj_x, prj_y, prj_z]        (5 x M, moving)
  in [128 n x 2048 m] PSUM groups (4 banks).  One DVE tensor_tensor_reduce
  per group drains PSUM to SBUF bf16 (relu-fused via max(d2, 0)) while
  min-reducing over m into a per-(n-tile, m-group) rowmin slot; a second
  DVE tensor_tensor folds the group into a running bf16 colmin buffer
  (elementwise min across n-tiles).  Padded fg rows (sentinel 10000.0)
  produce d2 ~ 3e8, so they never win either min; the rowmin contribution
  of padded rows is zeroed by a host-built (mask/L) multiply.
  Finally colmin [128, 4096] is min-reduced across partitions with 32
  TensorE transposes + one DVE min-reduce, and both chamfer terms are
  summed across partitions with a K=128 matmul against a ones vector.
"""

import sys

sys.path.insert(0, "/opt/trn_rl_repo")
sys.path.insert(0, "/root/.axon_site/_ro/trn_rl_repo")

import numpy as np

import concourse.bass as bass
import concourse.mybir as mybir
import concourse.tile as tile
from concourse.masks import make_identity

B, N, M, D = 8, 4096, 4096, 3
PAD = 10000.0
P = 128  # partitions / n-tile rows
MG = 2048  # m elements per PSUM group (4 banks)
N_TILES = N // P  # 32
N_GROUPS = M // MG  # 2
BIG = 1.0e30

_cached = {}


def _patch_tile_commit_waits():
    """This walrus build rejects >1 sync-wait per instruction: hoist extra
    waits onto nofuse NOPs committed just before the instruction on the same
    engine (engine streams are in-order, so prefix waits are equivalent)."""
    if getattr(tile.TileContext, "_wait_split_patched", False):
        return
    orig_commit = tile.TileContext._commit_instruction

    def _commit_split(self, inst, lazy_reg_writes=True):
        si = getattr(inst, "sync_info", None)
        eng = getattr(inst, "engine", None)
        if (
            si is not None
            and si.on_wait
            and len(si.on_wait) > 1
            and eng is not None
            and eng != mybir.EngineType.Unassigned
        ):
            waits = list(si.on_wait)
            si.on_wait = waits[:1]
            for w in waits[1:]:
                nop = mybir.InstNoOp(
                    name=f"I-{self.nc.next_id()}",
                    sync_info=mybir.SyncInfo(on_wait=[w], on_update=[]),
                    bass_nofuse=True,
                    engine=eng,
                )
                orig_commit(self, nop, lazy_reg_writes=False)
        return orig_commit(self, inst, lazy_reg_writes)

    tile.TileContext._commit_instruction = _commit_split
    tile.TileContext._wait_split_patched = True


def _patch_tile_tail_drain():
    """This walrus build rejects >1 sync-wait on a TPB_CTRL (Drain)
    instruction; split the TileContext tail-drain's wait list across a chain
    of single-wait drains on the sync engine."""
    from bass_rust import ScopedClock

    def _drain_and_barrier(self, tick_clock, wait_clock):
        nc = self.nc
        drain_inst = nc.sync.drain()
        wait_clock.add_sem_waits(
            drain_inst.ins, ScopedClock({None: tick_clock.global_clock})
        )
        si = drain_inst.ins.sync_info
        waits = list(si.on_wait) if si is not None and si.on_wait else []
        if len(waits) > 1:
            si.on_wait = waits[:1]
            for w in waits[1:]:
                extra = nc.sync.drain()
                esi = extra.ins.sync_info
                if esi is None:
                    extra.ins.sync_info = type(si)(on_wait=[w], on_update=[])
                else:
                    esi.on_wait = [w]

        nc.all_engine_barrier()
        assert self.sems is not None
        popped = nc._tile_sem_poison_stack.pop()
        assert popped is self._sem_poison
        nc.clear_and_free_semaphores(list(self.sems.allocated().values()))
        nc.all_engine_barrier()

    tile.TileContext._drain_and_barrier = _drain_and_barrier


def _build_program():
    _patch_tile_commit_waits()
    _patch_tile_tail_drain()
    f32 = mybir.dt.float32
    bf16 = mybir.dt.bfloat16
    Alu = mybir.AluOpType
    Ax = mybir.AxisListType

    nc = bass.Bass("TRN2", target_bir_lowering=False, debug=False, num_devices=B)
    fg_aug = nc.dram_tensor("fg_aug", [16, N], bf16, kind="ExternalInput").ap()
    prj_aug = nc.dram_tensor("prj_aug", [16, M], bf16, kind="ExternalInput").ap()
    mask = nc.dram_tensor("mask", [P, N_TILES], f32, kind="ExternalInput").ap()
    out = nc.dram_tensor("out", [1, 1], f32, kind="ExternalOutput").ap()

    with tile.TileContext(nc) as tc:
        with (
            tc.tile_pool(name="consts", bufs=1) as consts,
            tc.tile_pool(name="d2p", bufs=4) as d2p,
            tc.tile_pool(name="foldp", bufs=3) as foldp,
            tc.tile_pool(name="psum", bufs=2, space="PSUM") as psum,
        ):
            fg_sb = consts.tile([16, N], bf16)
            prj_sb = consts.tile([16, M], bf16)
            mask_sb = consts.tile([P, N_TILES], f32)
            nc.sync.dma_start(out=fg_sb[:], in_=fg_aug)
            nc.sync.dma_start(out=prj_sb[:], in_=prj_aug)
            nc.sync.dma_start(out=mask_sb[:], in_=mask)

            ones_sb = consts.tile([P, 1], f32)
            nc.vector.memset(ones_sb[:], 1.0)
            ident_sb = consts.tile([P, P], bf16)
            make_identity(nc, ident_sb)

            colmin = consts.tile([P, M], bf16)
            nc.gpsimd.memset(colmin[:], BIG)
            rowmin_parts = consts.tile([P, N_TILES, N_GROUPS], f32)

            # ---- main loop: 32 n-tiles x 2 m-groups ----
            for i in range(N_TILES):
                lhsT = fg_sb[:, i * P : (i + 1) * P]
                for g in range(N_GROUPS):
                    grp = psum.tile([P, MG], f32, tag="grp")
                    for j in range(MG // 512):
                        m0 = g * MG + j * 512
                        # d2 = fg2 + prj2 - 2 fg.prj at ~f32 precision: one
                        # K=16 bf16 matmul over round-to-nearest hi/lo splits
                        # (all hi- and lo-cross-terms in the contraction).
                        nc.tensor.matmul(
                            grp[:, j * 512 : (j + 1) * 512],
                            lhsT,
                            prj_sb[:, m0 : m0 + 512],
                            start=True,
                            stop=True,
                        )
                    d2 = d2p.tile([P, MG], bf16)
                    # drain PSUM -> SBUF bf16 with fused relu on ScalarE
                    nc.scalar.activation(
                        d2[:], grp[:], mybir.ActivationFunctionType.Relu
                    )
                    # colmin slice = min(colmin slice, d2)
                    cslice = colmin[:, g * MG : (g + 1) * MG]
                    nc.vector.tensor_tensor(cslice, d2[:], cslice, Alu.min)
                    # rowmin: two 2x-mode TT-min folds, then a small 1x reduce
                    h1 = foldp.tile([P, MG // 2], bf16)
                    nc.vector.tensor_tensor(
                        h1[:], d2[:, : MG // 2], d2[:, MG // 2 :], Alu.min
                    )
                    nc.vector.tensor_tensor(
                        h1[:, : MG // 4], h1[:, : MG // 4], h1[:, MG // 4 :], Alu.min
                    )
                    nc.vector.tensor_tensor(
                        h1[:, : MG // 8], h1[:, : MG // 8], h1[:, MG // 8 : MG // 4], Alu.min
                    )
                    nc.vector.tensor_reduce(
                        rowmin_parts[:, i, g : g + 1],
                        h1[:, : MG // 8],
                        axis=Ax.X,
                        op=Alu.min,
                    )

            # ---- cham_x: rowmin -> masked mean over valid rows ----
            rowmin2 = consts.tile([P, N_TILES], f32)
            nc.vector.tensor_reduce(
                rowmin2[:], rowmin_parts[:], axis=Ax.X, op=Alu.min
            )
            rowx = consts.tile([P, N_TILES], f32)
            # rowx = max(rowmin2, 0) * (mask/L)
            nc.vector.scalar_tensor_tensor(
                rowx[:], rowmin2[:], 0.0, mask_sb[:], op0=Alu.max, op1=Alu.mult
            )

            # ---- cham_y: colmin across partitions via PE transposes ----
            tp = psum.tile([P, N_TILES, P], bf16, tag="grp")
            for bank in range(4):
                for k in range(8):
                    c = bank * 8 + k
                    nc.tensor.transpose(
                        tp[:, c, :],
                        colmin[:, c * P : (c + 1) * P],
                        ident_sb[:],
                    )
            colmin2 = consts.tile([P, N_TILES], f32)
            nc.vector.tensor_reduce(colmin2[:], tp[:], axis=Ax.X, op=Alu.min)
            coly = consts.tile([P, N_TILES], f32)
            # coly = max(colmin2, 0) * (1/M)
            nc.vector.tensor_scalar(
                out=coly[:],
                in0=colmin2[:],
                scalar1=0.0,
                scalar2=1.0 / M,
                op0=Alu.max,
                op1=Alu.mult,
            )

            # ---- total = sum_p sum_i (rowx + coly) via K=128 matmul ----
            tot32 = consts.tile([P, N_TILES], f32)
            nc.vector.tensor_add(tot32[:], rowx[:], coly[:])
            tot = consts.tile([P, 1], f32)
            nc.vector.tensor_reduce(tot[:], tot32[:], axis=Ax.X, op=Alu.add)
            px = psum.tile([1, 512], f32, tag="grp")
            nc.tensor.matmul(px[:, 0:1], tot[:], ones_sb[:], start=True, stop=True)
            res = consts.tile([1, 1], f32)
            nc.vector.tensor_copy(out=res[:], in_=px[:, 0:1])
            nc.sync.dma_start(out=out, in_=res[:])

    return nc


def _split_bf16(x):
    """Round-to-nearest bf16 hi/lo split: x ~= hi + lo to ~16 mantissa bits."""
    import ml_dtypes

    hi = x.astype(np.float32).astype(ml_dtypes.bfloat16)
    lo = (x.astype(np.float32) - hi.astype(np.float32)).astype(ml_dtypes.bfloat16)
    return hi, lo


def _prep_core_inputs(fg, prj, length):
    """Host-side prep for one sample: hi/lo-split augmented matmul operands.

    MM1 (K=10): lhsT=[a_hi,a_lo,1,1,-2fh,-2fl] rhs=[1,1,b_hi,b_lo,ph,ph]
      -> fg2 + prj2 - 2 fg . p_hi
    MM2 (K=6):  lhsT=[-2fh,-2fl]               rhs=[pl,pl]
      -> -2 fg . p_lo   (PSUM-accumulated onto MM1)
    """
    import ml_dtypes

    bf = ml_dtypes.bfloat16
    f = fg.astype(np.float32)
    p = prj.astype(np.float32)
    L = int(length)
    fg2 = (f.astype(np.float64) ** 2).sum(-1).astype(np.float32)
    prj2 = (p.astype(np.float64) ** 2).sum(-1).astype(np.float32)
    a_hi, a_lo = _split_bf16(fg2)
    b_hi, b_lo = _split_bf16(prj2)
    f_hi, f_lo = _split_bf16(f)  # [N, 3] each
    p_hi, p_lo = _split_bf16(p)  # [M, 3] each
    ones_n = np.ones(N, bf)
    ones_m = np.ones(M, bf)
    f2_hi = (-2.0 * f_hi.astype(np.float32)).astype(bf)  # exact scale by -2
    f2_lo = (-2.0 * f_lo.astype(np.float32)).astype(bf)
    fg_aug = np.ascontiguousarray(
        np.stack(
            [a_hi, a_lo, ones_n, ones_n]
            + [f2_hi[:, d] for d in range(3)]
            + [f2_lo[:, d] for d in range(3)]
            + [f2_hi[:, d] for d in range(3)]
            + [f2_lo[:, d] for d in range(3)]
        )
    )
    prj_aug = np.ascontiguousarray(
        np.stack(
            [ones_m, ones_m, b_hi, b_lo]
            + [p_hi[:, d] for d in range(3)]
            + [p_hi[:, d] for d in range(3)]
            + [p_lo[:, d] for d in range(3)]
            + [p_lo[:, d] for d in range(3)]
        )
    )
    mask = (np.arange(N) < L).astype(np.float32).reshape(N_TILES, P).T / L
    return {
        "fg_aug": fg_aug,
        "prj_aug": prj_aug,
        "mask": np.ascontiguousarray(mask),
    }


def _run(in_maps, trace=False):
    from concourse.bass_utils import run_bass_kernel_spmd

    if "nc" not in _cached:
        _cached["nc"] = _build_program()
    return run_bass_kernel_spmd(
        _cached["nc"], in_maps, list(range(B)), trace=trace
    )


def kernel(fg_points, prj_points, x_lengths, _trace=False):
    fg = np.asarray(fg_points)
    prj = np.asarray(prj_points)
    lengths = np.asarray(x_lengths)
    in_maps = [
        _prep_core_inputs(fg[b], prj[b], lengths[b]) for b in range(B)
    ]
    res = _run(in_maps, trace=_trace)
    vals = [float(res.results[b]["out"][0, 0]) for b in range(B)]
    out = np.array(np.mean(vals), dtype=np.float32)
    if _trace:
        return out, res
    return out


# revision 18
# speedup vs baseline: 4.0266x; 1.6533x over previous
"""Chamfer distance loss on 8 Trainium2 NeuronCores.

Work-balanced sharding: only n-tiles containing valid (non-PAD) fg rows
contribute to the loss, so the kernel builds a schedule from x_lengths at
call time: every sample's valid n-tile range is cut into 4-tile *segments*
(512 rows each) and the segments are distributed evenly across the 8 cores
(SPMD: every core runs the same program over SEG_PER_CORE segments; dummy
segments are fed sentinel rows and ignored by the host).

Per tile (128 valid-ish fg rows x full 4096 prj):
  d2[n, m] = |fg_n|^2 + |prj_m|^2 - 2 fg_n . prj_m  is ONE K=16 bf16
  matmul per 512-wide PSUM bank over round-to-nearest bf16 hi/lo splits
  of the augmented operands (all hi/lo cross terms in the contraction ->
  ~f32 accuracy at full bf16 PE speed).  ScalarE drains each [128 x 2048]
  PSUM group to SBUF bf16 with fused relu; VectorE folds the group into a
  per-segment running colmin buffer (2x-mode tensor_tensor min) and
  reduces the row direction with a 3-level 2x fold + small 1x reduce.

The per-core outputs (per-segment colmin partials [128, 4096] and per-
(tile, m-group) rowmin partials) are combined on the host: elementwise
min across segments of the same sample, min across the 128 tile rows,
relu, and the masked means.  PAD rows produce d2 ~ 3e8 and never win a
min; their rowmin contribution is dropped by the host mask.
"""

import sys

sys.path.insert(0, "/opt/trn_rl_repo")
sys.path.insert(0, "/root/.axon_site/_ro/trn_rl_repo")

import numpy as np

import concourse.bass as bass
import concourse.mybir as mybir
import concourse.tile as tile

B, N, M, D = 8, 4096, 4096, 3
PAD = 10000.0
P = 128  # partitions / rows per n-tile
MG = 2048  # m elements per PSUM group (4 banks)
N_GROUPS = M // MG  # 2
TILES_PER_SEG = 4  # n-tiles per schedule segment (512 rows)
BIG = 1.0e30

_cached = {}


def _patch_tile_commit_waits():
    """This walrus build rejects >1 sync-wait per instruction: hoist extra
    waits onto nofuse NOPs committed just before the instruction on the same
    engine (engine streams are in-order, so prefix waits are equivalent)."""
    if getattr(tile.TileContext, "_wait_split_patched", False):
        return
    orig_commit = tile.TileContext._commit_instruction

    def _commit_split(self, inst, lazy_reg_writes=True):
        si = getattr(inst, "sync_info", None)
        eng = getattr(inst, "engine", None)
        if (
            si is not None
            and si.on_wait
            and len(si.on_wait) > 1
            and eng is not None
            and eng != mybir.EngineType.Unassigned
        ):
            waits = list(si.on_wait)
            si.on_wait = waits[:1]
            for w in waits[1:]:
                nop = mybir.InstNoOp(
                    name=f"I-{self.nc.next_id()}",
                    sync_info=mybir.SyncInfo(on_wait=[w], on_update=[]),
                    bass_nofuse=True,
                    engine=eng,
                )
                orig_commit(self, nop, lazy_reg_writes=False)
        return orig_commit(self, inst, lazy_reg_writes)

    tile.TileContext._commit_instruction = _commit_split
    tile.TileContext._wait_split_patched = True


def _patch_tile_tail_drain():
    """This walrus build rejects >1 sync-wait on a TPB_CTRL (Drain)
    instruction; split the TileContext tail-drain's wait list across a chain
    of single-wait drains on the sync engine."""
    from bass_rust import ScopedClock

    def _drain_and_barrier(self, tick_clock, wait_clock):
        nc = self.nc
        drain_inst = nc.sync.drain()
        wait_clock.add_sem_waits(
            drain_inst.ins, ScopedClock({None: tick_clock.global_clock})
        )
        si = drain_inst.ins.sync_info
        waits = list(si.on_wait) if si is not None and si.on_wait else []
        if len(waits) > 1:
            si.on_wait = waits[:1]
            for w in waits[1:]:
                extra = nc.sync.drain()
                esi = extra.ins.sync_info
                if esi is None:
                    extra.ins.sync_info = type(si)(on_wait=[w], on_update=[])
                else:
                    esi.on_wait = [w]

        nc.all_engine_barrier()
        assert self.sems is not None
        popped = nc._tile_sem_poison_stack.pop()
        assert popped is self._sem_poison
        nc.clear_and_free_semaphores(list(self.sems.allocated().values()))
        nc.all_engine_barrier()

    tile.TileContext._drain_and_barrier = _drain_and_barrier


def _build_program(seg_per_core):
    _patch_tile_commit_waits()
    _patch_tile_tail_drain()
    f32 = mybir.dt.float32
    bf16 = mybir.dt.bfloat16
    Alu = mybir.AluOpType
    Ax = mybir.AxisListType

    slots = seg_per_core * TILES_PER_SEG  # n-tile slots per core
    nc = bass.Bass("TRN2", target_bir_lowering=False, debug=False, num_devices=B)
    fg_in = nc.dram_tensor("fg_c", [16, slots * P], bf16, kind="ExternalInput").ap()
    prj_in = nc.dram_tensor(
        "prj_c", [16, seg_per_core * M], bf16, kind="ExternalInput"
    ).ap()
    colmin_out = nc.dram_tensor(
        "colmin_out", [P, seg_per_core * M], bf16, kind="ExternalOutput"
    ).ap()
    rowmin_out = nc.dram_tensor(
        "rowmin_out", [P, slots * N_GROUPS], f32, kind="ExternalOutput"
    ).ap()

    with tile.TileContext(nc) as tc:
        with (
            tc.tile_pool(name="consts", bufs=1) as consts,
            tc.tile_pool(name="d2p", bufs=4) as d2p,
            tc.tile_pool(name="foldp", bufs=3) as foldp,
            tc.tile_pool(name="psum", bufs=2, space="PSUM") as psum,
        ):
            fg_sb = consts.tile([16, slots * P], bf16)
            prj_sb = consts.tile([16, seg_per_core * M], bf16)
            nc.sync.dma_start(out=fg_sb[:], in_=fg_in)
            nc.sync.dma_start(out=prj_sb[:], in_=prj_in)

            colmin = consts.tile([P, seg_per_core * M], bf16)
            nc.gpsimd.memset(colmin[:], BIG)
            rowmin_parts = consts.tile([P, slots, N_GROUPS], f32)

            for t in range(slots):
                seg = t // TILES_PER_SEG
                lhsT = fg_sb[:, t * P : (t + 1) * P]
                for g in range(N_GROUPS):
                    grp = psum.tile([P, MG], f32, tag="grp")
                    for j in range(MG // 512):
                        m0 = seg * M + g * MG + j * 512
                        nc.tensor.matmul(
                            grp[:, j * 512 : (j + 1) * 512],
                            lhsT,
                            prj_sb[:, m0 : m0 + 512],
                            start=True,
                            stop=True,
                        )
                    d2 = d2p.tile([P, MG], bf16)
                    # drain PSUM -> SBUF bf16 with fused relu on ScalarE
                    nc.scalar.activation(
                        d2[:], grp[:], mybir.ActivationFunctionType.Relu
                    )
                    # segment colmin slice = min(colmin slice, d2)
                    c0 = seg * M + g * MG
                    cslice = colmin[:, c0 : c0 + MG]
                    nc.vector.tensor_tensor(cslice, d2[:], cslice, Alu.min)
                    # rowmin: three 2x-mode TT-min folds + small 1x reduce
                    h1 = foldp.tile([P, MG // 2], bf16)
                    nc.vector.tensor_tensor(
                        h1[:], d2[:, : MG // 2], d2[:, MG // 2 :], Alu.min
                    )
                    nc.vector.tensor_tensor(
                        h1[:, : MG // 4], h1[:, : MG // 4], h1[:, MG // 4 :], Alu.min
                    )
                    nc.vector.tensor_tensor(
                        h1[:, : MG // 8],
                        h1[:, : MG // 8],
                        h1[:, MG // 8 : MG // 4],
                        Alu.min,
                    )
                    nc.vector.tensor_reduce(
                        rowmin_parts[:, t, g : g + 1],
                        h1[:, : MG // 8],
                        axis=Ax.X,
                        op=Alu.min,
                    )

            nc.sync.dma_start(out=colmin_out, in_=colmin[:])
            nc.sync.dma_start(
                out=rowmin_out, in_=rowmin_parts[:].rearrange("p t g -> p (t g)")
            )

    return nc


def _split_bf16(x):
    """Round-to-nearest bf16 hi/lo split: x ~= hi + lo to ~16 mantissa bits."""
    import ml_dtypes

    hi = x.astype(np.float32).astype(ml_dtypes.bfloat16)
    lo = (x.astype(np.float32) - hi.astype(np.float32)).astype(ml_dtypes.bfloat16)
    return hi, lo


def _aug16(pts, sq):
    """[16, n] bf16 lhsT-side augmentation rows for d2 via one K=16 matmul:
    [a_hi, a_lo, 1, 1, -2f_hi(3), -2f_lo(3), -2f_hi(3), -2f_lo(3)]."""
    import ml_dtypes

    bf = ml_dtypes.bfloat16
    n = pts.shape[0]
    a_hi, a_lo = _split_bf16(sq)
    f_hi, f_lo = _split_bf16(pts)
    f2_hi = (-2.0 * f_hi.astype(np.float32)).astype(bf)
    f2_lo = (-2.0 * f_lo.astype(np.float32)).astype(bf)
    ones = np.ones(n, bf)
    return np.stack(
        [a_hi, a_lo, ones, ones]
        + [f2_hi[:, d] for d in range(3)]
        + [f2_lo[:, d] for d in range(3)]
        + [f2_hi[:, d] for d in range(3)]
        + [f2_lo[:, d] for d in range(3)]
    )


def _aug16_rhs(pts, sq):
    """[16, m] bf16 rhs-side augmentation rows:
    [1, 1, b_hi, b_lo, p_hi(3), p_hi(3), p_lo(3), p_lo(3)]."""
    import ml_dtypes

    bf = ml_dtypes.bfloat16
    m = pts.shape[0]
    b_hi, b_lo = _split_bf16(sq)
    p_hi, p_lo = _split_bf16(pts)
    ones = np.ones(m, bf)
    return np.stack(
        [ones, ones, b_hi, b_lo]
        + [p_hi[:, d] for d in range(3)]
        + [p_hi[:, d] for d in range(3)]
        + [p_lo[:, d] for d in range(3)]
        + [p_lo[:, d] for d in range(3)]
    )


def _build_schedule(lengths):
    """Split every sample's valid n-tile range into 4-tile segments and pack
    them into 8 equal per-core lists (padded with dummy segments)."""
    segs = []  # (sample, first_tile)
    for s in range(B):
        ntiles = max(1, -(-int(lengths[s]) // P))  # ceil(L/128), >= 1
        for st in range(0, ntiles, TILES_PER_SEG):
            segs.append((s, st))
    seg_per_core = -(-len(segs) // B)
    while len(segs) < seg_per_core * B:
        segs.append(None)  # dummy
    cores = [segs[c * seg_per_core : (c + 1) * seg_per_core] for c in range(B)]
    return cores, seg_per_core


def _prep_inputs(fg, prj, lengths):
    import ml_dtypes

    bf = ml_dtypes.bfloat16
    cores, seg_per_core = _build_schedule(lengths)
    slots = seg_per_core * TILES_PER_SEG

    fg_f = fg.astype(np.float32)
    prj_f = prj.astype(np.float32)
    fg2 = (fg_f.astype(np.float64) ** 2).sum(-1).astype(np.float32)
    prj2 = (prj_f.astype(np.float64) ** 2).sum(-1).astype(np.float32)
    fg_aug = {s: _aug16(fg_f[s], fg2[s]) for s in range(B)}  # [16, N]
    prj_aug = {s: _aug16_rhs(prj_f[s], prj2[s]) for s in range(B)}  # [16, M]
    pad_cols = np.full((16, P * TILES_PER_SEG), 0, bf)
    pad_cols[0, :] = bf(BIG)  # d2 of dummy rows = BIG + prj2 - 0 >> any real d2

    in_maps = []
    for c in range(B):
        fg_c = np.empty((16, slots * P), bf)
        prj_c = np.empty((16, seg_per_core * M), bf)
        for k, seg in enumerate(cores[c]):
            lo = k * TILES_PER_SEG * P
            hi = lo + TILES_PER_SEG * P
            if seg is None:
                fg_c[:, lo:hi] = pad_cols
                prj_c[:, k * M : (k + 1) * M] = prj_aug[0]
            else:
                s, st = seg
                r0 = st * P
                fg_c[:, lo:hi] = fg_aug[s][:, r0 : r0 + TILES_PER_SEG * P]
                prj_c[:, k * M : (k + 1) * M] = prj_aug[s]
        in_maps.append(
            {"fg_c": np.ascontiguousarray(fg_c), "prj_c": np.ascontiguousarray(prj_c)}
        )
    return in_maps, cores, seg_per_core


def _combine(results, cores, seg_per_core, lengths):
    """Host-side reduction of the per-core partials to the scalar loss."""
    colmin = {}  # sample -> running [P, M] f32 min
    rowsum = np.zeros(B, np.float64)  # per-sample masked sum of rowmins
    for c in range(B):
        cm = np.asarray(results[c]["colmin_out"], dtype=np.float32)
        rm = np.asarray(results[c]["rowmin_out"], dtype=np.float32)
        cm = cm.reshape(P, seg_per_core, M)
        rm = rm.reshape(P, seg_per_core * TILES_PER_SEG, N_GROUPS).min(axis=2)
        for k, seg in enumerate(cores[c]):
            if seg is None:
                continue
            s, st = seg
            prev = colmin.get(s)
            cur = cm[:, k, :]
            colmin[s] = cur if prev is None else np.minimum(prev, cur)
            L = int(lengths[s])
            for tt in range(TILES_PER_SEG):
                n0 = (st + tt) * P
                nvalid = min(max(L - n0, 0), P)
                if nvalid <= 0:
                    continue
                rmin = rm[:nvalid, k * TILES_PER_SEG + tt]
                rowsum[s] += np.maximum(rmin, 0.0).sum()
    total = 0.0
    for s in range(B):
        L = int(lengths[s])
        cham_x = rowsum[s] / L
        cham_y = np.maximum(colmin[s].min(axis=0), 0.0).mean()
        total += cham_x + cham_y
    return np.float32(total / B)


def _run(in_maps, seg_per_core, trace=False):
    from concourse.bass_utils import run_bass_kernel_spmd

    key = ("nc", seg_per_core)
    if key not in _cached:
        _cached[key] = _build_program(seg_per_core)
    return run_bass_kernel_spmd(_cached[key], in_maps, list(range(B)), trace=trace)


def kernel(fg_points, prj_points, x_lengths, _trace=False):
    fg = np.asarray(fg_points)
    prj = np.asarray(prj_points)
    lengths = np.asarray(x_lengths)
    in_maps, cores, seg_per_core = _prep_inputs(fg, prj, lengths)
    res = _run(in_maps, seg_per_core, trace=_trace)
    out = _combine(res.results, cores, seg_per_core, lengths)
    if _trace:
        return out, res
    return out


# revision 19
# speedup vs baseline: 4.5591x; 1.1323x over previous
"""Chamfer distance loss on 8 Trainium2 NeuronCores.

Work-balanced sharding: only n-tiles containing valid (non-PAD) fg rows
contribute to the loss, so the kernel builds a schedule from x_lengths at
call time: every sample's valid n-tile range is cut into 4-tile *segments*
(512 rows each) and the segments are distributed evenly across the 8 cores
(SPMD: every core runs the same program over SEG_PER_CORE segments; dummy
segments are fed sentinel rows and ignored by the host).

Per tile (128 valid-ish fg rows x full 4096 prj):
  d2[n, m] = |fg_n|^2 + |prj_m|^2 - 2 fg_n . prj_m  is ONE K=16 bf16
  matmul per 512-wide PSUM bank over round-to-nearest bf16 hi/lo splits
  of the augmented operands (all hi/lo cross terms in the contraction ->
  ~f32 accuracy at full bf16 PE speed).  ScalarE drains each [128 x 2048]
  PSUM group to SBUF bf16 with fused relu; VectorE folds the group into a
  per-segment running colmin buffer (2x-mode tensor_tensor min) and
  reduces the row direction with a 3-level 2x fold + small 1x reduce.

The per-core outputs (per-segment colmin partials [128, 4096] and per-
(tile, m-group) rowmin partials) are combined on the host: elementwise
min across segments of the same sample, min across the 128 tile rows,
relu, and the masked means.  PAD rows produce d2 ~ 3e8 and never win a
min; their rowmin contribution is dropped by the host mask.
"""

import sys

sys.path.insert(0, "/opt/trn_rl_repo")
sys.path.insert(0, "/root/.axon_site/_ro/trn_rl_repo")

import numpy as np

import concourse.bass as bass
import concourse.mybir as mybir
import concourse.tile as tile

B, N, M, D = 8, 4096, 4096, 3
PAD = 10000.0
P = 128  # partitions / rows per n-tile
MG = 2048  # m elements per PSUM group (4 banks)
N_GROUPS = M // MG  # 2
TILES_PER_SEG = 4  # n-tiles per schedule segment (512 rows)
BIG = 1.0e30

_cached = {}


def _patch_tile_commit_waits():
    """This walrus build rejects >1 sync-wait per instruction: hoist extra
    waits onto nofuse NOPs committed just before the instruction on the same
    engine (engine streams are in-order, so prefix waits are equivalent)."""
    if getattr(tile.TileContext, "_wait_split_patched", False):
        return
    orig_commit = tile.TileContext._commit_instruction

    def _commit_split(self, inst, lazy_reg_writes=True):
        si = getattr(inst, "sync_info", None)
        eng = getattr(inst, "engine", None)
        if (
            si is not None
            and si.on_wait
            and len(si.on_wait) > 1
            and eng is not None
            and eng != mybir.EngineType.Unassigned
        ):
            waits = list(si.on_wait)
            si.on_wait = waits[:1]
            for w in waits[1:]:
                nop = mybir.InstNoOp(
                    name=f"I-{self.nc.next_id()}",
                    sync_info=mybir.SyncInfo(on_wait=[w], on_update=[]),
                    bass_nofuse=True,
                    engine=eng,
                )
                orig_commit(self, nop, lazy_reg_writes=False)
        return orig_commit(self, inst, lazy_reg_writes)

    tile.TileContext._commit_instruction = _commit_split
    tile.TileContext._wait_split_patched = True


def _patch_tile_tail_drain():
    """This walrus build rejects >1 sync-wait on a TPB_CTRL (Drain)
    instruction; split the TileContext tail-drain's wait list across a chain
    of single-wait drains on the sync engine."""
    from bass_rust import ScopedClock

    def _drain_and_barrier(self, tick_clock, wait_clock):
        nc = self.nc
        drain_inst = nc.sync.drain()
        wait_clock.add_sem_waits(
            drain_inst.ins, ScopedClock({None: tick_clock.global_clock})
        )
        si = drain_inst.ins.sync_info
        waits = list(si.on_wait) if si is not None and si.on_wait else []
        if len(waits) > 1:
            si.on_wait = waits[:1]
            for w in waits[1:]:
                extra = nc.sync.drain()
                esi = extra.ins.sync_info
                if esi is None:
                    extra.ins.sync_info = type(si)(on_wait=[w], on_update=[])
                else:
                    esi.on_wait = [w]

        nc.all_engine_barrier()
        assert self.sems is not None
        popped = nc._tile_sem_poison_stack.pop()
        assert popped is self._sem_poison
        nc.clear_and_free_semaphores(list(self.sems.allocated().values()))
        nc.all_engine_barrier()

    tile.TileContext._drain_and_barrier = _drain_and_barrier


def _build_program(seg_per_core):
    _patch_tile_commit_waits()
    _patch_tile_tail_drain()
    f32 = mybir.dt.float32
    bf16 = mybir.dt.bfloat16
    Alu = mybir.AluOpType
    Ax = mybir.AxisListType

    slots = seg_per_core * TILES_PER_SEG  # n-tile slots per core
    nc = bass.Bass("TRN2", target_bir_lowering=False, debug=False, num_devices=B)
    fg_in = nc.dram_tensor("fg_c", [16, slots * P], bf16, kind="ExternalInput").ap()
    prj_in = nc.dram_tensor(
        "prj_c", [16, seg_per_core * M], bf16, kind="ExternalInput"
    ).ap()
    colmin_out = nc.dram_tensor(
        "colmin_out", [P, seg_per_core * M], bf16, kind="ExternalOutput"
    ).ap()
    rowmin_out = nc.dram_tensor(
        "rowmin_out", [P, slots * N_GROUPS], f32, kind="ExternalOutput"
    ).ap()

    with tile.TileContext(nc) as tc:
        with (
            tc.tile_pool(name="consts", bufs=1) as consts,
            tc.tile_pool(name="d2p", bufs=4) as d2p,
            tc.tile_pool(name="foldp", bufs=3) as foldp,
            tc.tile_pool(name="psum", bufs=2, space="PSUM") as psum,
        ):
            fg_sb = consts.tile([16, slots * P], bf16)
            prj_sb = consts.tile([16, seg_per_core * M], bf16)
            nc.sync.dma_start(out=fg_sb[:], in_=fg_in)
            nc.sync.dma_start(out=prj_sb[:], in_=prj_in)

            colmin = consts.tile([P, seg_per_core * M], bf16)
            rowmin_parts = consts.tile([P, slots, N_GROUPS], f32)

            for t in range(slots):
                seg = t // TILES_PER_SEG
                lhsT = fg_sb[:, t * P : (t + 1) * P]
                for g in range(N_GROUPS):
                    grp = psum.tile([P, MG], f32, tag="grp")
                    for j in range(MG // 512):
                        m0 = seg * M + g * MG + j * 512
                        nc.tensor.matmul(
                            grp[:, j * 512 : (j + 1) * 512],
                            lhsT,
                            prj_sb[:, m0 : m0 + 512],
                            start=True,
                            stop=True,
                        )
                    d2 = d2p.tile([P, MG], bf16)
                    # drain PSUM -> SBUF bf16 with fused relu on ScalarE
                    nc.scalar.activation(
                        d2[:], grp[:], mybir.ActivationFunctionType.Relu
                    )
                    # segment colmin slice: first tile of the segment copies
                    # (4x-mode), later tiles fold in with 2x-mode min
                    c0 = seg * M + g * MG
                    cslice = colmin[:, c0 : c0 + MG]
                    if t % TILES_PER_SEG == 0:
                        nc.vector.tensor_copy(out=cslice, in_=d2[:])
                    else:
                        nc.vector.tensor_tensor(cslice, d2[:], cslice, Alu.min)
                    # rowmin: three 2x-mode TT-min folds + small 1x reduce
                    h1 = foldp.tile([P, MG // 2], bf16)
                    nc.vector.tensor_tensor(
                        h1[:], d2[:, : MG // 2], d2[:, MG // 2 :], Alu.min
                    )
                    nc.vector.tensor_tensor(
                        h1[:, : MG // 4], h1[:, : MG // 4], h1[:, MG // 4 :], Alu.min
                    )
                    nc.vector.tensor_tensor(
                        h1[:, : MG // 8],
                        h1[:, : MG // 8],
                        h1[:, MG // 8 : MG // 4],
                        Alu.min,
                    )
                    nc.vector.tensor_reduce(
                        rowmin_parts[:, t, g : g + 1],
                        h1[:, : MG // 8],
                        axis=Ax.X,
                        op=Alu.min,
                    )
                # stream each finished segment's colmin back to DRAM
                if t % TILES_PER_SEG == TILES_PER_SEG - 1:
                    nc.sync.dma_start(
                        out=colmin_out[:, seg * M : (seg + 1) * M],
                        in_=colmin[:, seg * M : (seg + 1) * M],
                    )

            nc.sync.dma_start(
                out=rowmin_out, in_=rowmin_parts[:].rearrange("p t g -> p (t g)")
            )

    return nc


def _split_bf16(x):
    """Round-to-nearest bf16 hi/lo split: x ~= hi + lo to ~16 mantissa bits."""
    import ml_dtypes

    hi = x.astype(np.float32).astype(ml_dtypes.bfloat16)
    lo = (x.astype(np.float32) - hi.astype(np.float32)).astype(ml_dtypes.bfloat16)
    return hi, lo


def _aug16(pts, sq):
    """[16, n] bf16 lhsT-side augmentation rows for d2 via one K=16 matmul:
    [a_hi, a_lo, 1, 1, -2f_hi(3), -2f_lo(3), -2f_hi(3), -2f_lo(3)]."""
    import ml_dtypes

    bf = ml_dtypes.bfloat16
    n = pts.shape[0]
    a_hi, a_lo = _split_bf16(sq)
    f_hi, f_lo = _split_bf16(pts)
    f2_hi = (-2.0 * f_hi.astype(np.float32)).astype(bf)
    f2_lo = (-2.0 * f_lo.astype(np.float32)).astype(bf)
    ones = np.ones(n, bf)
    return np.stack(
        [a_hi, a_lo, ones, ones]
        + [f2_hi[:, d] for d in range(3)]
        + [f2_lo[:, d] for d in range(3)]
        + [f2_hi[:, d] for d in range(3)]
        + [f2_lo[:, d] for d in range(3)]
    )


def _aug16_rhs(pts, sq):
    """[16, m] bf16 rhs-side augmentation rows:
    [1, 1, b_hi, b_lo, p_hi(3), p_hi(3), p_lo(3), p_lo(3)]."""
    import ml_dtypes

    bf = ml_dtypes.bfloat16
    m = pts.shape[0]
    b_hi, b_lo = _split_bf16(sq)
    p_hi, p_lo = _split_bf16(pts)
    ones = np.ones(m, bf)
    return np.stack(
        [ones, ones, b_hi, b_lo]
        + [p_hi[:, d] for d in range(3)]
        + [p_hi[:, d] for d in range(3)]
        + [p_lo[:, d] for d in range(3)]
        + [p_lo[:, d] for d in range(3)]
    )


def _build_schedule(lengths):
    """Split every sample's valid n-tile range into 4-tile segments and pack
    them into 8 equal per-core lists (padded with dummy segments)."""
    segs = []  # (sample, first_tile)
    for s in range(B):
        ntiles = max(1, -(-int(lengths[s]) // P))  # ceil(L/128), >= 1
        for st in range(0, ntiles, TILES_PER_SEG):
            segs.append((s, st))
    seg_per_core = -(-len(segs) // B)
    while len(segs) < seg_per_core * B:
        segs.append(None)  # dummy
    cores = [segs[c * seg_per_core : (c + 1) * seg_per_core] for c in range(B)]
    return cores, seg_per_core


def _prep_inputs(fg, prj, lengths):
    import ml_dtypes

    bf = ml_dtypes.bfloat16
    cores, seg_per_core = _build_schedule(lengths)
    slots = seg_per_core * TILES_PER_SEG

    fg_f = fg.astype(np.float32)
    prj_f = prj.astype(np.float32)
    fg2 = (fg_f.astype(np.float64) ** 2).sum(-1).astype(np.float32)
    prj2 = (prj_f.astype(np.float64) ** 2).sum(-1).astype(np.float32)
    fg_aug = {s: _aug16(fg_f[s], fg2[s]) for s in range(B)}  # [16, N]
    prj_aug = {s: _aug16_rhs(prj_f[s], prj2[s]) for s in range(B)}  # [16, M]
    pad_cols = np.full((16, P * TILES_PER_SEG), 0, bf)
    pad_cols[0, :] = bf(BIG)  # d2 of dummy rows = BIG + prj2 - 0 >> any real d2

    in_maps = []
    for c in range(B):
        fg_c = np.empty((16, slots * P), bf)
        prj_c = np.empty((16, seg_per_core * M), bf)
        for k, seg in enumerate(cores[c]):
            lo = k * TILES_PER_SEG * P
            hi = lo + TILES_PER_SEG * P
            if seg is None:
                fg_c[:, lo:hi] = pad_cols
                prj_c[:, k * M : (k + 1) * M] = prj_aug[0]
            else:
                s, st = seg
                r0 = st * P
                fg_c[:, lo:hi] = fg_aug[s][:, r0 : r0 + TILES_PER_SEG * P]
                prj_c[:, k * M : (k + 1) * M] = prj_aug[s]
        in_maps.append(
            {"fg_c": np.ascontiguousarray(fg_c), "prj_c": np.ascontiguousarray(prj_c)}
        )
    return in_maps, cores, seg_per_core


def _combine(results, cores, seg_per_core, lengths):
    """Host-side reduction of the per-core partials to the scalar loss."""
    colmin = {}  # sample -> running [P, M] f32 min
    rowsum = np.zeros(B, np.float64)  # per-sample masked sum of rowmins
    for c in range(B):
        cm = np.asarray(results[c]["colmin_out"], dtype=np.float32)
        rm = np.asarray(results[c]["rowmin_out"], dtype=np.float32)
        cm = cm.reshape(P, seg_per_core, M)
        rm = rm.reshape(P, seg_per_core * TILES_PER_SEG, N_GROUPS).min(axis=2)
        for k, seg in enumerate(cores[c]):
            if seg is None:
                continue
            s, st = seg
            prev = colmin.get(s)
            cur = cm[:, k, :]
            colmin[s] = cur if prev is None else np.minimum(prev, cur)
            L = int(lengths[s])
            for tt in range(TILES_PER_SEG):
                n0 = (st + tt) * P
                nvalid = min(max(L - n0, 0), P)
                if nvalid <= 0:
                    continue
                rmin = rm[:nvalid, k * TILES_PER_SEG + tt]
                rowsum[s] += np.maximum(rmin, 0.0).sum()
    total = 0.0
    for s in range(B):
        L = int(lengths[s])
        cham_x = rowsum[s] / L
        cham_y = np.maximum(colmin[s].min(axis=0), 0.0).mean()
        total += cham_x + cham_y
    return np.float32(total / B)


def _run(in_maps, seg_per_core, trace=False):
    from concourse.bass_utils import run_bass_kernel_spmd

    key = ("nc", seg_per_core)
    if key not in _cached:
        _cached[key] = _build_program(seg_per_core)
    return run_bass_kernel_spmd(_cached[key], in_maps, list(range(B)), trace=trace)


def kernel(fg_points, prj_points, x_lengths, _trace=False):
    fg = np.asarray(fg_points)
    prj = np.asarray(prj_points)
    lengths = np.asarray(x_lengths)
    in_maps, cores, seg_per_core = _prep_inputs(fg, prj, lengths)
    res = _run(in_maps, seg_per_core, trace=_trace)
    out = _combine(res.results, cores, seg_per_core, lengths)
    if _trace:
        return out, res
    return out


# revision 20
# speedup vs baseline: 4.8981x; 1.0743x over previous
"""Chamfer distance loss on 8 Trainium2 NeuronCores.

Work-balanced sharding: only n-tiles containing valid (non-PAD) fg rows
contribute to the loss, so the kernel builds a schedule from x_lengths at
call time: every sample's valid n-tile range is cut into 4-tile *segments*
(512 rows each) and the segments are distributed evenly across the 8 cores
(SPMD: every core runs the same program over SEG_PER_CORE segments; dummy
segments are fed sentinel rows and ignored by the host).

Per tile (128 valid-ish fg rows x full 4096 prj):
  d2[n, m] = |fg_n|^2 + |prj_m|^2 - 2 fg_n . prj_m  is ONE K=16 bf16
  matmul per 512-wide PSUM bank over round-to-nearest bf16 hi/lo splits
  of the augmented operands (all hi/lo cross terms in the contraction ->
  ~f32 accuracy at full bf16 PE speed).  ScalarE drains each [128 x 2048]
  PSUM group to SBUF bf16 with fused relu; VectorE folds the group into a
  per-segment running colmin buffer (2x-mode tensor_tensor min) and
  reduces the row direction with a 3-level 2x fold + small 1x reduce.

The per-core outputs (per-segment colmin partials [128, 4096] and per-
(tile, m-group) rowmin partials) are combined on the host: elementwise
min across segments of the same sample, min across the 128 tile rows,
relu, and the masked means.  PAD rows produce d2 ~ 3e8 and never win a
min; their rowmin contribution is dropped by the host mask.
"""

import sys

sys.path.insert(0, "/opt/trn_rl_repo")
sys.path.insert(0, "/root/.axon_site/_ro/trn_rl_repo")

import numpy as np

import concourse.bass as bass
import concourse.mybir as mybir
import concourse.tile as tile

B, N, M, D = 8, 4096, 4096, 3
PAD = 10000.0
P = 128  # partitions / rows per n-tile
MG = 2048  # m elements per PSUM group (4 banks)
N_GROUPS = M // MG  # 2
TILES_PER_SEG = 4  # n-tiles per schedule segment (512 rows)
BIG = 1.0e30

_cached = {}


def _patch_tile_commit_waits():
    """This walrus build rejects >1 sync-wait per instruction: hoist extra
    waits onto nofuse NOPs committed just before the instruction on the same
    engine (engine streams are in-order, so prefix waits are equivalent)."""
    if getattr(tile.TileContext, "_wait_split_patched", False):
        return
    orig_commit = tile.TileContext._commit_instruction

    def _commit_split(self, inst, lazy_reg_writes=True):
        si = getattr(inst, "sync_info", None)
        eng = getattr(inst, "engine", None)
        if (
            si is not None
            and si.on_wait
            and len(si.on_wait) > 1
            and eng is not None
            and eng != mybir.EngineType.Unassigned
        ):
            waits = list(si.on_wait)
            si.on_wait = waits[:1]
            for w in waits[1:]:
                nop = mybir.InstNoOp(
                    name=f"I-{self.nc.next_id()}",
                    sync_info=mybir.SyncInfo(on_wait=[w], on_update=[]),
                    bass_nofuse=True,
                    engine=eng,
                )
                orig_commit(self, nop, lazy_reg_writes=False)
        return orig_commit(self, inst, lazy_reg_writes)

    tile.TileContext._commit_instruction = _commit_split
    tile.TileContext._wait_split_patched = True


def _patch_tile_tail_drain():
    """This walrus build rejects >1 sync-wait on a TPB_CTRL (Drain)
    instruction; split the TileContext tail-drain's wait list across a chain
    of single-wait drains on the sync engine."""
    from bass_rust import ScopedClock

    def _drain_and_barrier(self, tick_clock, wait_clock):
        nc = self.nc
        drain_inst = nc.sync.drain()
        wait_clock.add_sem_waits(
            drain_inst.ins, ScopedClock({None: tick_clock.global_clock})
        )
        si = drain_inst.ins.sync_info
        waits = list(si.on_wait) if si is not None and si.on_wait else []
        if len(waits) > 1:
            si.on_wait = waits[:1]
            for w in waits[1:]:
                extra = nc.sync.drain()
                esi = extra.ins.sync_info
                if esi is None:
                    extra.ins.sync_info = type(si)(on_wait=[w], on_update=[])
                else:
                    esi.on_wait = [w]

        nc.all_engine_barrier()
        assert self.sems is not None
        popped = nc._tile_sem_poison_stack.pop()
        assert popped is self._sem_poison
        nc.clear_and_free_semaphores(list(self.sems.allocated().values()))
        nc.all_engine_barrier()

    tile.TileContext._drain_and_barrier = _drain_and_barrier


def _build_program(seg_per_core):
    _patch_tile_commit_waits()
    _patch_tile_tail_drain()
    f32 = mybir.dt.float32
    bf16 = mybir.dt.bfloat16
    Alu = mybir.AluOpType
    Ax = mybir.AxisListType

    slots = seg_per_core * TILES_PER_SEG  # n-tile slots per core
    nc = bass.Bass("TRN2", target_bir_lowering=False, debug=False, num_devices=B)
    fg_in = nc.dram_tensor("fg_c", [16, slots * P], bf16, kind="ExternalInput").ap()
    prj_in = nc.dram_tensor(
        "prj_c", [16, seg_per_core * M], bf16, kind="ExternalInput"
    ).ap()
    colmin_out = nc.dram_tensor(
        "colmin_out", [P, seg_per_core * M], bf16, kind="ExternalOutput"
    ).ap()
    rowmin_out = nc.dram_tensor(
        "rowmin_out", [P, slots * N_GROUPS], f32, kind="ExternalOutput"
    ).ap()

    with tile.TileContext(nc) as tc:
        with (
            tc.tile_pool(name="consts", bufs=1) as consts,
            tc.tile_pool(name="d2p", bufs=4) as d2p,
            tc.tile_pool(name="foldp", bufs=3) as foldp,
            tc.tile_pool(name="psum", bufs=2, space="PSUM") as psum,
        ):
            fg_sb = consts.tile([16, slots * P], bf16)
            prj_sb = consts.tile([16, seg_per_core * M], bf16)
            # per-segment chunks so segment 0's matmuls start early (the
            # 16-partition layout only drives 2 of 16 DMA ports)
            for k in range(seg_per_core):
                f0, f1 = k * TILES_PER_SEG * P, (k + 1) * TILES_PER_SEG * P
                nc.sync.dma_start(out=fg_sb[:, f0:f1], in_=fg_in[:, f0:f1])
                nc.sync.dma_start(
                    out=prj_sb[:, k * M : (k + 1) * M],
                    in_=prj_in[:, k * M : (k + 1) * M],
                )

            colmin = consts.tile([P, seg_per_core * M], bf16)
            rowmin_parts = consts.tile([P, slots, N_GROUPS], f32)

            for t in range(slots):
                seg = t // TILES_PER_SEG
                lhsT = fg_sb[:, t * P : (t + 1) * P]
                for g in range(N_GROUPS):
                    grp = psum.tile([P, MG], f32, tag="grp")
                    for j in range(MG // 512):
                        m0 = seg * M + g * MG + j * 512
                        nc.tensor.matmul(
                            grp[:, j * 512 : (j + 1) * 512],
                            lhsT,
                            prj_sb[:, m0 : m0 + 512],
                            start=True,
                            stop=True,
                        )
                    c0 = seg * M + g * MG
                    cslice = colmin[:, c0 : c0 + MG]
                    if t % TILES_PER_SEG == 0:
                        # first tile of the segment: ScalarE drains straight
                        # into the colmin slice (no DVE copy needed)
                        d2 = cslice
                        nc.scalar.activation(
                            d2, grp[:], mybir.ActivationFunctionType.Relu
                        )
                    else:
                        d2t = d2p.tile([P, MG], bf16)
                        d2 = d2t[:]
                        # drain PSUM -> SBUF bf16 with fused relu on ScalarE
                        nc.scalar.activation(
                            d2, grp[:], mybir.ActivationFunctionType.Relu
                        )
                        nc.vector.tensor_tensor(cslice, d2, cslice, Alu.min)
                    # rowmin: three 2x-mode TT-min folds + small 1x reduce
                    h1 = foldp.tile([P, MG // 2], bf16)
                    nc.vector.tensor_tensor(
                        h1[:], d2[:, : MG // 2], d2[:, MG // 2 :], Alu.min
                    )
                    nc.vector.tensor_tensor(
                        h1[:, : MG // 4], h1[:, : MG // 4], h1[:, MG // 4 :], Alu.min
                    )
                    nc.vector.tensor_tensor(
                        h1[:, : MG // 8],
                        h1[:, : MG // 8],
                        h1[:, MG // 8 : MG // 4],
                        Alu.min,
                    )
                    nc.vector.tensor_reduce(
                        rowmin_parts[:, t, g : g + 1],
                        h1[:, : MG // 8],
                        axis=Ax.X,
                        op=Alu.min,
                    )
                # stream each finished segment's colmin back to DRAM
                if t % TILES_PER_SEG == TILES_PER_SEG - 1:
                    nc.sync.dma_start(
                        out=colmin_out[:, seg * M : (seg + 1) * M],
                        in_=colmin[:, seg * M : (seg + 1) * M],
                    )

            nc.sync.dma_start(
                out=rowmin_out, in_=rowmin_parts[:].rearrange("p t g -> p (t g)")
            )

    return nc


def _split_bf16(x):
    """Round-to-nearest bf16 hi/lo split: x ~= hi + lo to ~16 mantissa bits."""
    import ml_dtypes

    hi = x.astype(np.float32).astype(ml_dtypes.bfloat16)
    lo = (x.astype(np.float32) - hi.astype(np.float32)).astype(ml_dtypes.bfloat16)
    return hi, lo


def _aug16(pts, sq):
    """[16, n] bf16 lhsT-side augmentation rows for d2 via one K=16 matmul:
    [a_hi, a_lo, 1, 1, -2f_hi(3), -2f_lo(3), -2f_hi(3), -2f_lo(3)]."""
    import ml_dtypes

    bf = ml_dtypes.bfloat16
    n = pts.shape[0]
    a_hi, a_lo = _split_bf16(sq)
    f_hi, f_lo = _split_bf16(pts)
    f2_hi = (-2.0 * f_hi.astype(np.float32)).astype(bf)
    f2_lo = (-2.0 * f_lo.astype(np.float32)).astype(bf)
    ones = np.ones(n, bf)
    return np.stack(
        [a_hi, a_lo, ones, ones]
        + [f2_hi[:, d] for d in range(3)]
        + [f2_lo[:, d] for d in range(3)]
        + [f2_hi[:, d] for d in range(3)]
        + [f2_lo[:, d] for d in range(3)]
    )


def _aug16_rhs(pts, sq):
    """[16, m] bf16 rhs-side augmentation rows:
    [1, 1, b_hi, b_lo, p_hi(3), p_hi(3), p_lo(3), p_lo(3)]."""
    import ml_dtypes

    bf = ml_dtypes.bfloat16
    m = pts.shape[0]
    b_hi, b_lo = _split_bf16(sq)
    p_hi, p_lo = _split_bf16(pts)
    ones = np.ones(m, bf)
    return np.stack(
        [ones, ones, b_hi, b_lo]
        + [p_hi[:, d] for d in range(3)]
        + [p_hi[:, d] for d in range(3)]
        + [p_lo[:, d] for d in range(3)]
        + [p_lo[:, d] for d in range(3)]
    )


def _build_schedule(lengths):
    """Split every sample's valid n-tile range into 4-tile segments and pack
    them into 8 equal per-core lists (padded with dummy segments)."""
    segs = []  # (sample, first_tile)
    for s in range(B):
        ntiles = max(1, -(-int(lengths[s]) // P))  # ceil(L/128), >= 1
        for st in range(0, ntiles, TILES_PER_SEG):
            segs.append((s, st))
    seg_per_core = -(-len(segs) // B)
    while len(segs) < seg_per_core * B:
        segs.append(None)  # dummy
    cores = [segs[c * seg_per_core : (c + 1) * seg_per_core] for c in range(B)]
    return cores, seg_per_core


def _prep_inputs(fg, prj, lengths):
    import ml_dtypes

    bf = ml_dtypes.bfloat16
    cores, seg_per_core = _build_schedule(lengths)
    slots = seg_per_core * TILES_PER_SEG

    fg_f = fg.astype(np.float32)
    prj_f = prj.astype(np.float32)
    fg2 = (fg_f.astype(np.float64) ** 2).sum(-1).astype(np.float32)
    prj2 = (prj_f.astype(np.float64) ** 2).sum(-1).astype(np.float32)
    fg_aug = {s: _aug16(fg_f[s], fg2[s]) for s in range(B)}  # [16, N]
    prj_aug = {s: _aug16_rhs(prj_f[s], prj2[s]) for s in range(B)}  # [16, M]
    pad_cols = np.full((16, P * TILES_PER_SEG), 0, bf)
    pad_cols[0, :] = bf(BIG)  # d2 of dummy rows = BIG + prj2 - 0 >> any real d2

    in_maps = []
    for c in range(B):
        fg_c = np.empty((16, slots * P), bf)
        prj_c = np.empty((16, seg_per_core * M), bf)
        for k, seg in enumerate(cores[c]):
            lo = k * TILES_PER_SEG * P
            hi = lo + TILES_PER_SEG * P
            if seg is None:
                fg_c[:, lo:hi] = pad_cols
                prj_c[:, k * M : (k + 1) * M] = prj_aug[0]
            else:
                s, st = seg
                r0 = st * P
                fg_c[:, lo:hi] = fg_aug[s][:, r0 : r0 + TILES_PER_SEG * P]
                prj_c[:, k * M : (k + 1) * M] = prj_aug[s]
        in_maps.append(
            {"fg_c": np.ascontiguousarray(fg_c), "prj_c": np.ascontiguousarray(prj_c)}
        )
    return in_maps, cores, seg_per_core


def _combine(results, cores, seg_per_core, lengths):
    """Host-side reduction of the per-core partials to the scalar loss."""
    colmin = {}  # sample -> running [P, M] f32 min
    rowsum = np.zeros(B, np.float64)  # per-sample masked sum of rowmins
    for c in range(B):
        cm = np.asarray(results[c]["colmin_out"], dtype=np.float32)
        rm = np.asarray(results[c]["rowmin_out"], dtype=np.float32)
        cm = cm.reshape(P, seg_per_core, M)
        rm = rm.reshape(P, seg_per_core * TILES_PER_SEG, N_GROUPS).min(axis=2)
        for k, seg in enumerate(cores[c]):
            if seg is None:
                continue
            s, st = seg
            prev = colmin.get(s)
            cur = cm[:, k, :]
            colmin[s] = cur if prev is None else np.minimum(prev, cur)
            L = int(lengths[s])
            for tt in range(TILES_PER_SEG):
                n0 = (st + tt) * P
                nvalid = min(max(L - n0, 0), P)
                if nvalid <= 0:
                    continue
                rmin = rm[:nvalid, k * TILES_PER_SEG + tt]
                rowsum[s] += np.maximum(rmin, 0.0).sum()
    total = 0.0
    for s in range(B):
        L = int(lengths[s])
        cham_x = rowsum[s] / L
        cham_y = np.maximum(colmin[s].min(axis=0), 0.0).mean()
        total += cham_x + cham_y
    return np.float32(total / B)


def _run(in_maps, seg_per_core, trace=False):
    from concourse.bass_utils import run_bass_kernel_spmd

    key = ("nc", seg_per_core)
    if key not in _cached:
        _cached[key] = _build_program(seg_per_core)
    return run_bass_kernel_spmd(_cached[key], in_maps, list(range(B)), trace=trace)


def kernel(fg_points, prj_points, x_lengths, _trace=False):
    fg = np.asarray(fg_points)
    prj = np.asarray(prj_points)
    lengths = np.asarray(x_lengths)
    in_maps, cores, seg_per_core = _prep_inputs(fg, prj, lengths)
    res = _run(in_maps, seg_per_core, trace=_trace)
    out = _combine(res.results, cores, seg_per_core, lengths)
    if _trace:
        return out, res
    return out


# revision 21
# speedup vs baseline: 4.9832x; 1.0174x over previous
"""Chamfer distance loss on 8 Trainium2 NeuronCores.

Work-balanced sharding: only n-tiles containing valid (non-PAD) fg rows
contribute to the loss, so the kernel builds a schedule from x_lengths at
call time: every sample's valid n-tile range is cut into 4-tile *segments*
(512 rows each) and the segments are distributed evenly across the 8 cores
(SPMD: every core runs the same program over SEG_PER_CORE segments; dummy
segments are fed sentinel rows and ignored by the host).

Per tile (128 valid-ish fg rows x full 4096 prj):
  d2[n, m] = |fg_n|^2 + |prj_m|^2 - 2 fg_n . prj_m  is ONE K=16 bf16
  matmul per 512-wide PSUM bank over round-to-nearest bf16 hi/lo splits
  of the augmented operands (all hi/lo cross terms in the contraction ->
  ~f32 accuracy at full bf16 PE speed).  ScalarE drains each [128 x 2048]
  PSUM group to SBUF bf16 with fused relu; VectorE folds the group into a
  per-segment running colmin buffer (2x-mode tensor_tensor min) and
  reduces the row direction with a 3-level 2x fold + small 1x reduce.

The per-core outputs (per-segment colmin partials [128, 4096] and per-
(tile, m-group) rowmin partials) are combined on the host: elementwise
min across segments of the same sample, min across the 128 tile rows,
relu, and the masked means.  PAD rows produce d2 ~ 3e8 and never win a
min; their rowmin contribution is dropped by the host mask.
"""

import sys

sys.path.insert(0, "/opt/trn_rl_repo")
sys.path.insert(0, "/root/.axon_site/_ro/trn_rl_repo")

import numpy as np

import concourse.bass as bass
import concourse.mybir as mybir
import concourse.tile as tile

B, N, M, D = 8, 4096, 4096, 3
PAD = 10000.0
P = 128  # partitions / rows per n-tile
MG = 2048  # m elements per PSUM group (4 banks)
N_GROUPS = M // MG  # 2
TILES_PER_SEG = 4  # n-tiles per schedule segment (512 rows)
BIG = 1.0e30

_cached = {}


def _patch_tile_commit_waits():
    """This walrus build rejects >1 sync-wait per instruction: hoist extra
    waits onto nofuse NOPs committed just before the instruction on the same
    engine (engine streams are in-order, so prefix waits are equivalent)."""
    if getattr(tile.TileContext, "_wait_split_patched", False):
        return
    orig_commit = tile.TileContext._commit_instruction

    def _commit_split(self, inst, lazy_reg_writes=True):
        si = getattr(inst, "sync_info", None)
        eng = getattr(inst, "engine", None)
        if (
            si is not None
            and si.on_wait
            and len(si.on_wait) > 1
            and eng is not None
            and eng != mybir.EngineType.Unassigned
        ):
            waits = list(si.on_wait)
            si.on_wait = waits[:1]
            for w in waits[1:]:
                nop = mybir.InstNoOp(
                    name=f"I-{self.nc.next_id()}",
                    sync_info=mybir.SyncInfo(on_wait=[w], on_update=[]),
                    bass_nofuse=True,
                    engine=eng,
                )
                orig_commit(self, nop, lazy_reg_writes=False)
        return orig_commit(self, inst, lazy_reg_writes)

    tile.TileContext._commit_instruction = _commit_split
    tile.TileContext._wait_split_patched = True


def _patch_tile_tail_drain():
    """This walrus build rejects >1 sync-wait on a TPB_CTRL (Drain)
    instruction; split the TileContext tail-drain's wait list across a chain
    of single-wait drains on the sync engine."""
    from bass_rust import ScopedClock

    def _drain_and_barrier(self, tick_clock, wait_clock):
        nc = self.nc
        drain_inst = nc.sync.drain()
        wait_clock.add_sem_waits(
            drain_inst.ins, ScopedClock({None: tick_clock.global_clock})
        )
        si = drain_inst.ins.sync_info
        waits = list(si.on_wait) if si is not None and si.on_wait else []
        if len(waits) > 1:
            si.on_wait = waits[:1]
            for w in waits[1:]:
                extra = nc.sync.drain()
                esi = extra.ins.sync_info
                if esi is None:
                    extra.ins.sync_info = type(si)(on_wait=[w], on_update=[])
                else:
                    esi.on_wait = [w]

        nc.all_engine_barrier()
        assert self.sems is not None
        popped = nc._tile_sem_poison_stack.pop()
        assert popped is self._sem_poison
        nc.clear_and_free_semaphores(list(self.sems.allocated().values()))
        nc.all_engine_barrier()

    tile.TileContext._drain_and_barrier = _drain_and_barrier


def _build_program(seg_per_core):
    _patch_tile_commit_waits()
    _patch_tile_tail_drain()
    f32 = mybir.dt.float32
    bf16 = mybir.dt.bfloat16
    Alu = mybir.AluOpType
    Ax = mybir.AxisListType

    slots = seg_per_core * TILES_PER_SEG  # n-tile slots per core
    nc = bass.Bass("TRN2", target_bir_lowering=False, debug=False, num_devices=B)
    fg_in = nc.dram_tensor("fg_c", [16, slots * P], bf16, kind="ExternalInput").ap()
    prj_in = nc.dram_tensor(
        "prj_c", [16, seg_per_core * M], bf16, kind="ExternalInput"
    ).ap()
    colmin_out = nc.dram_tensor(
        "colmin_out", [P, seg_per_core * M], bf16, kind="ExternalOutput"
    ).ap()
    rowmin_out = nc.dram_tensor(
        "rowmin_out", [P, slots * N_GROUPS], f32, kind="ExternalOutput"
    ).ap()

    with tile.TileContext(nc) as tc:
        with (
            tc.tile_pool(name="consts", bufs=1) as consts,
            tc.tile_pool(name="d2p", bufs=4) as d2p,
            tc.tile_pool(name="foldp", bufs=3) as foldp,
            tc.tile_pool(name="psum", bufs=2, space="PSUM") as psum,
        ):
            fg_sb = consts.tile([16, slots * P], bf16)
            prj_sb = consts.tile([16, seg_per_core * M], bf16)
            # per-segment chunks so segment 0's matmuls start early (the
            # 16-partition layout only drives 2 of 16 DMA ports)
            for k in range(seg_per_core):
                f0, f1 = k * TILES_PER_SEG * P, (k + 1) * TILES_PER_SEG * P
                nc.sync.dma_start(out=fg_sb[:, f0:f1], in_=fg_in[:, f0:f1])
                if k == 0:
                    nc.sync.dma_start(out=prj_sb[:, :MG], in_=prj_in[:, :MG])
                    nc.sync.dma_start(out=prj_sb[:, MG:M], in_=prj_in[:, MG:M])
                else:
                    nc.sync.dma_start(
                        out=prj_sb[:, k * M : (k + 1) * M],
                        in_=prj_in[:, k * M : (k + 1) * M],
                    )

            colmin = consts.tile([P, seg_per_core * M], bf16)
            rowmin_parts = consts.tile([P, slots, N_GROUPS], f32)
            # pre-load ScalarE's activation table while input DMAs stream
            warm = consts.tile([P, 1], f32)
            nc.vector.memset(warm[:], 0.0)
            nc.scalar.activation(warm[:], warm[:], mybir.ActivationFunctionType.Relu)

            for t in range(slots):
                seg = t // TILES_PER_SEG
                lhsT = fg_sb[:, t * P : (t + 1) * P]
                for g in range(N_GROUPS):
                    grp = psum.tile([P, MG], f32, tag="grp")
                    for j in range(MG // 512):
                        m0 = seg * M + g * MG + j * 512
                        nc.tensor.matmul(
                            grp[:, j * 512 : (j + 1) * 512],
                            lhsT,
                            prj_sb[:, m0 : m0 + 512],
                            start=True,
                            stop=True,
                        )
                    c0 = seg * M + g * MG
                    cslice = colmin[:, c0 : c0 + MG]
                    if t % TILES_PER_SEG == 0:
                        # first tile of the segment: ScalarE drains straight
                        # into the colmin slice (no DVE copy needed)
                        d2 = cslice
                        nc.scalar.activation(
                            d2, grp[:], mybir.ActivationFunctionType.Relu
                        )
                    else:
                        d2t = d2p.tile([P, MG], bf16)
                        d2 = d2t[:]
                        # drain PSUM -> SBUF bf16 with fused relu on ScalarE
                        nc.scalar.activation(
                            d2, grp[:], mybir.ActivationFunctionType.Relu
                        )
                        nc.vector.tensor_tensor(cslice, d2, cslice, Alu.min)
                    # rowmin: three 2x-mode TT-min folds + small 1x reduce
                    h1 = foldp.tile([P, MG // 2], bf16)
                    nc.vector.tensor_tensor(
                        h1[:], d2[:, : MG // 2], d2[:, MG // 2 :], Alu.min
                    )
                    nc.vector.tensor_tensor(
                        h1[:, : MG // 4], h1[:, : MG // 4], h1[:, MG // 4 :], Alu.min
                    )
                    nc.vector.tensor_tensor(
                        h1[:, : MG // 8],
                        h1[:, : MG // 8],
                        h1[:, MG // 8 : MG // 4],
                        Alu.min,
                    )
                    nc.vector.tensor_reduce(
                        rowmin_parts[:, t, g : g + 1],
                        h1[:, : MG // 8],
                        axis=Ax.X,
                        op=Alu.min,
                    )
                # stream each finished segment's colmin back to DRAM
                if t % TILES_PER_SEG == TILES_PER_SEG - 1:
                    for gg in range(N_GROUPS):
                        o0 = seg * M + gg * MG
                        nc.sync.dma_start(
                            out=colmin_out[:, o0 : o0 + MG],
                            in_=colmin[:, o0 : o0 + MG],
                        )

            nc.sync.dma_start(
                out=rowmin_out, in_=rowmin_parts[:].rearrange("p t g -> p (t g)")
            )

    return nc


def _split_bf16(x):
    """Round-to-nearest bf16 hi/lo split: x ~= hi + lo to ~16 mantissa bits."""
    import ml_dtypes

    hi = x.astype(np.float32).astype(ml_dtypes.bfloat16)
    lo = (x.astype(np.float32) - hi.astype(np.float32)).astype(ml_dtypes.bfloat16)
    return hi, lo


def _aug16(pts, sq):
    """[16, n] bf16 lhsT-side augmentation rows for d2 via one K=16 matmul:
    [a_hi, a_lo, 1, 1, -2f_hi(3), -2f_lo(3), -2f_hi(3), -2f_lo(3)]."""
    import ml_dtypes

    bf = ml_dtypes.bfloat16
    n = pts.shape[0]
    a_hi, a_lo = _split_bf16(sq)
    f_hi, f_lo = _split_bf16(pts)
    f2_hi = (-2.0 * f_hi.astype(np.float32)).astype(bf)
    f2_lo = (-2.0 * f_lo.astype(np.float32)).astype(bf)
    ones = np.ones(n, bf)
    return np.stack(
        [a_hi, a_lo, ones, ones]
        + [f2_hi[:, d] for d in range(3)]
        + [f2_lo[:, d] for d in range(3)]
        + [f2_hi[:, d] for d in range(3)]
        + [f2_lo[:, d] for d in range(3)]
    )


def _aug16_rhs(pts, sq):
    """[16, m] bf16 rhs-side augmentation rows:
    [1, 1, b_hi, b_lo, p_hi(3), p_hi(3), p_lo(3), p_lo(3)]."""
    import ml_dtypes

    bf = ml_dtypes.bfloat16
    m = pts.shape[0]
    b_hi, b_lo = _split_bf16(sq)
    p_hi, p_lo = _split_bf16(pts)
    ones = np.ones(m, bf)
    return np.stack(
        [ones, ones, b_hi, b_lo]
        + [p_hi[:, d] for d in range(3)]
        + [p_hi[:, d] for d in range(3)]
        + [p_lo[:, d] for d in range(3)]
        + [p_lo[:, d] for d in range(3)]
    )


def _build_schedule(lengths):
    """Split every sample's valid n-tile range into 4-tile segments and pack
    them into 8 equal per-core lists (padded with dummy segments)."""
    segs = []  # (sample, first_tile)
    for s in range(B):
        ntiles = max(1, -(-int(lengths[s]) // P))  # ceil(L/128), >= 1
        for st in range(0, ntiles, TILES_PER_SEG):
            segs.append((s, st))
    seg_per_core = -(-len(segs) // B)
    while len(segs) < seg_per_core * B:
        segs.append(None)  # dummy
    cores = [segs[c * seg_per_core : (c + 1) * seg_per_core] for c in range(B)]
    return cores, seg_per_core


def _prep_inputs(fg, prj, lengths):
    import ml_dtypes

    bf = ml_dtypes.bfloat16
    cores, seg_per_core = _build_schedule(lengths)
    slots = seg_per_core * TILES_PER_SEG

    fg_f = fg.astype(np.float32)
    prj_f = prj.astype(np.float32)
    fg2 = (fg_f.astype(np.float64) ** 2).sum(-1).astype(np.float32)
    prj2 = (prj_f.astype(np.float64) ** 2).sum(-1).astype(np.float32)
    fg_aug = {s: _aug16(fg_f[s], fg2[s]) for s in range(B)}  # [16, N]
    prj_aug = {s: _aug16_rhs(prj_f[s], prj2[s]) for s in range(B)}  # [16, M]
    pad_cols = np.full((16, P * TILES_PER_SEG), 0, bf)
    pad_cols[0, :] = bf(BIG)  # d2 of dummy rows = BIG + prj2 - 0 >> any real d2

    in_maps = []
    for c in range(B):
        fg_c = np.empty((16, slots * P), bf)
        prj_c = np.empty((16, seg_per_core * M), bf)
        for k, seg in enumerate(cores[c]):
            lo = k * TILES_PER_SEG * P
            hi = lo + TILES_PER_SEG * P
            if seg is None:
                fg_c[:, lo:hi] = pad_cols
                prj_c[:, k * M : (k + 1) * M] = prj_aug[0]
            else:
                s, st = seg
                r0 = st * P
                fg_c[:, lo:hi] = fg_aug[s][:, r0 : r0 + TILES_PER_SEG * P]
                prj_c[:, k * M : (k + 1) * M] = prj_aug[s]
        in_maps.append(
            {"fg_c": np.ascontiguousarray(fg_c), "prj_c": np.ascontiguousarray(prj_c)}
        )
    return in_maps, cores, seg_per_core


def _combine(results, cores, seg_per_core, lengths):
    """Host-side reduction of the per-core partials to the scalar loss."""
    colmin = {}  # sample -> running [P, M] f32 min
    rowsum = np.zeros(B, np.float64)  # per-sample masked sum of rowmins
    for c in range(B):
        cm = np.asarray(results[c]["colmin_out"], dtype=np.float32)
        rm = np.asarray(results[c]["rowmin_out"], dtype=np.float32)
        cm = cm.reshape(P, seg_per_core, M)
        rm = rm.reshape(P, seg_per_core * TILES_PER_SEG, N_GROUPS).min(axis=2)
        for k, seg in enumerate(cores[c]):
            if seg is None:
                continue
            s, st = seg
            prev = colmin.get(s)
            cur = cm[:, k, :]
            colmin[s] = cur if prev is None else np.minimum(prev, cur)
            L = int(lengths[s])
            for tt in range(TILES_PER_SEG):
                n0 = (st + tt) * P
                nvalid = min(max(L - n0, 0), P)
                if nvalid <= 0:
                    continue
                rmin = rm[:nvalid, k * TILES_PER_SEG + tt]
                rowsum[s] += np.maximum(rmin, 0.0).sum()
    total = 0.0
    for s in range(B):
        L = int(lengths[s])
        cham_x = rowsum[s] / L
        cham_y = np.maximum(colmin[s].min(axis=0), 0.0).mean()
        total += cham_x + cham_y
    return np.float32(total / B)


def _run(in_maps, seg_per_core, trace=False):
    from concourse.bass_utils import run_bass_kernel_spmd

    key = ("nc", seg_per_core)
    if key not in _cached:
        _cached[key] = _build_program(seg_per_core)
    return run_bass_kernel_spmd(_cached[key], in_maps, list(range(B)), trace=trace)


def kernel(fg_points, prj_points, x_lengths, _trace=False):
    fg = np.asarray(fg_points)
    prj = np.asarray(prj_points)
    lengths = np.asarray(x_lengths)
    in_maps, cores, seg_per_core = _prep_inputs(fg, prj, lengths)
    res = _run(in_maps, seg_per_core, trace=_trace)
    out = _combine(res.results, cores, seg_per_core, lengths)
    if _trace:
        return out, res
    return out


# revision 22
# speedup vs baseline: 5.1705x; 1.0376x over previous
"""Chamfer distance loss on 8 Trainium2 NeuronCores.

Work-balanced sharding: only n-tiles containing valid (non-PAD) fg rows
contribute to the loss, so the kernel builds a schedule from x_lengths at
call time: every sample's valid n-tile range is cut into 4-tile *segments*
(512 rows each) and the segments are distributed evenly across the 8 cores
(SPMD: every core runs the same program over SEG_PER_CORE segments; dummy
segments are fed sentinel rows and ignored by the host).

Per tile (128 valid-ish fg rows x full 4096 prj):
  d2[n, m] = |fg_n|^2 + |prj_m|^2 - 2 fg_n . prj_m  is ONE K=16 bf16
  matmul per 512-wide PSUM bank over round-to-nearest bf16 hi/lo splits
  of the augmented operands (all hi/lo cross terms in the contraction ->
  ~f32 accuracy at full bf16 PE speed).  ScalarE drains each [128 x 2048]
  PSUM group to SBUF bf16 with fused relu; VectorE folds the group into a
  per-segment running colmin buffer (2x-mode tensor_tensor min) and
  reduces the row direction with a 3-level 2x fold + small 1x reduce.

The per-core outputs (per-segment colmin partials [128, 4096] and per-
(tile, m-group) rowmin partials) are combined on the host: elementwise
min across segments of the same sample, min across the 128 tile rows,
relu, and the masked means.  PAD rows produce d2 ~ 3e8 and never win a
min; their rowmin contribution is dropped by the host mask.
"""

import sys

sys.path.insert(0, "/opt/trn_rl_repo")
sys.path.insert(0, "/root/.axon_site/_ro/trn_rl_repo")

import numpy as np

import concourse.bass as bass
import concourse.mybir as mybir
import concourse.tile as tile

B, N, M, D = 8, 4096, 4096, 3
PAD = 10000.0
P = 128  # partitions / rows per n-tile
MG = 2048  # m elements per PSUM group (4 banks)
N_GROUPS = M // MG  # 2
TILES_PER_SEG = 4  # n-tiles per schedule segment (512 rows)
BIG = 1.0e30

_cached = {}


def _patch_tile_commit_waits():
    """This walrus build rejects >1 sync-wait per instruction: hoist extra
    waits onto nofuse NOPs committed just before the instruction on the same
    engine (engine streams are in-order, so prefix waits are equivalent)."""
    if getattr(tile.TileContext, "_wait_split_patched", False):
        return
    orig_commit = tile.TileContext._commit_instruction

    def _commit_split(self, inst, lazy_reg_writes=True):
        si = getattr(inst, "sync_info", None)
        eng = getattr(inst, "engine", None)
        if (
            si is not None
            and si.on_wait
            and len(si.on_wait) > 1
            and eng is not None
            and eng != mybir.EngineType.Unassigned
        ):
            waits = list(si.on_wait)
            si.on_wait = waits[:1]
            for w in waits[1:]:
                nop = mybir.InstNoOp(
                    name=f"I-{self.nc.next_id()}",
                    sync_info=mybir.SyncInfo(on_wait=[w], on_update=[]),
                    bass_nofuse=True,
                    engine=eng,
                )
                orig_commit(self, nop, lazy_reg_writes=False)
        return orig_commit(self, inst, lazy_reg_writes)

    tile.TileContext._commit_instruction = _commit_split
    tile.TileContext._wait_split_patched = True


def _patch_tile_tail_drain():
    """This walrus build rejects >1 sync-wait on a TPB_CTRL (Drain)
    instruction; split the TileContext tail-drain's wait list across a chain
    of single-wait drains on the sync engine."""
    from bass_rust import ScopedClock

    def _drain_and_barrier(self, tick_clock, wait_clock):
        nc = self.nc
        drain_inst = nc.sync.drain()
        wait_clock.add_sem_waits(
            drain_inst.ins, ScopedClock({None: tick_clock.global_clock})
        )
        si = drain_inst.ins.sync_info
        waits = list(si.on_wait) if si is not None and si.on_wait else []
        if len(waits) > 1:
            si.on_wait = waits[:1]
            for w in waits[1:]:
                extra = nc.sync.drain()
                esi = extra.ins.sync_info
                if esi is None:
                    extra.ins.sync_info = type(si)(on_wait=[w], on_update=[])
                else:
                    esi.on_wait = [w]

        nc.all_engine_barrier()
        assert self.sems is not None
        popped = nc._tile_sem_poison_stack.pop()
        assert popped is self._sem_poison
        nc.clear_and_free_semaphores(list(self.sems.allocated().values()))
        nc.all_engine_barrier()

    tile.TileContext._drain_and_barrier = _drain_and_barrier


def _build_program(seg_per_core):
    _patch_tile_commit_waits()
    _patch_tile_tail_drain()
    f32 = mybir.dt.float32
    bf16 = mybir.dt.bfloat16
    Alu = mybir.AluOpType
    Ax = mybir.AxisListType

    slots = seg_per_core * TILES_PER_SEG  # n-tile slots per core
    nc = bass.Bass("TRN2", target_bir_lowering=False, debug=False, num_devices=B)
    fg_in = nc.dram_tensor("fg_c", [16, slots * P], bf16, kind="ExternalInput").ap()
    prj_in = nc.dram_tensor(
        "prj_c", [16, seg_per_core * M], bf16, kind="ExternalInput"
    ).ap()
    colmin_out = nc.dram_tensor(
        "colmin_out", [P, seg_per_core * M], bf16, kind="ExternalOutput"
    ).ap()
    rowmin_out = nc.dram_tensor(
        "rowmin_out", [P, slots], f32, kind="ExternalOutput"
    ).ap()

    with tile.TileContext(nc) as tc:
        with (
            tc.tile_pool(name="consts", bufs=1) as consts,
            tc.tile_pool(name="d2p", bufs=4) as d2p,
            tc.tile_pool(name="foldp", bufs=3) as foldp,
            tc.tile_pool(name="psum", bufs=2, space="PSUM") as psum,
        ):
            fg_sb = consts.tile([16, slots * P], bf16)
            prj_sb = consts.tile([16, seg_per_core * M], bf16)
            # per-segment chunks so segment 0's matmuls start early (the
            # 16-partition layout only drives 2 of 16 DMA ports)
            for k in range(seg_per_core):
                f0, f1 = k * TILES_PER_SEG * P, (k + 1) * TILES_PER_SEG * P
                nc.sync.dma_start(out=fg_sb[:, f0:f1], in_=fg_in[:, f0:f1])
                if k == 0:
                    nc.sync.dma_start(out=prj_sb[:, :MG], in_=prj_in[:, :MG])
                    nc.sync.dma_start(out=prj_sb[:, MG:M], in_=prj_in[:, MG:M])
                else:
                    nc.sync.dma_start(
                        out=prj_sb[:, k * M : (k + 1) * M],
                        in_=prj_in[:, k * M : (k + 1) * M],
                    )

            colmin = consts.tile([P, seg_per_core * M], bf16)
            rowmin_parts = consts.tile([P, slots], f32)
            # pre-load ScalarE's activation table while input DMAs stream
            warm = consts.tile([P, 1], f32)
            nc.vector.memset(warm[:], 0.0)
            nc.scalar.activation(warm[:], warm[:], mybir.ActivationFunctionType.Relu)

            for t in range(slots):
                seg = t // TILES_PER_SEG
                lhsT = fg_sb[:, t * P : (t + 1) * P]
                first = t % TILES_PER_SEG == 0
                c0 = seg * M
                cslice = colmin[:, c0 : c0 + M]
                if first:
                    # first tile of the segment: ScalarE drains straight
                    # into the colmin slice (no DVE copy needed)
                    d2 = cslice
                else:
                    d2t = d2p.tile([P, M], bf16)
                    d2 = d2t[:]
                for g in range(N_GROUPS):
                    grp = psum.tile([P, MG], f32, tag="grp")
                    for j in range(MG // 512):
                        m0 = seg * M + g * MG + j * 512
                        nc.tensor.matmul(
                            grp[:, j * 512 : (j + 1) * 512],
                            lhsT,
                            prj_sb[:, m0 : m0 + 512],
                            start=True,
                            stop=True,
                        )
                    # drain PSUM -> SBUF bf16 with fused relu on ScalarE
                    nc.scalar.activation(
                        d2[:, g * MG : (g + 1) * MG],
                        grp[:],
                        mybir.ActivationFunctionType.Relu,
                    )
                # whole-tile DVE ops (one op constant instead of two)
                if not first:
                    nc.vector.tensor_tensor(cslice, d2, cslice, Alu.min)
                # rowmin: four 2x-mode TT-min folds + small 1x reduce
                h1 = foldp.tile([P, M // 2], bf16)
                nc.vector.tensor_tensor(h1[:], d2[:, : M // 2], d2[:, M // 2 :], Alu.min)
                nc.vector.tensor_tensor(
                    h1[:, : M // 4], h1[:, : M // 4], h1[:, M // 4 :], Alu.min
                )
                nc.vector.tensor_tensor(
                    h1[:, : M // 8], h1[:, : M // 8], h1[:, M // 8 : M // 4], Alu.min
                )
                nc.vector.tensor_tensor(
                    h1[:, : M // 16], h1[:, : M // 16], h1[:, M // 16 : M // 8], Alu.min
                )
                nc.vector.tensor_reduce(
                    rowmin_parts[:, t : t + 1],
                    h1[:, : M // 16],
                    axis=Ax.X,
                    op=Alu.min,
                )
                # stream each finished segment's colmin back to DRAM
                if t % TILES_PER_SEG == TILES_PER_SEG - 1:
                    for gg in range(N_GROUPS):
                        o0 = seg * M + gg * MG
                        nc.sync.dma_start(
                            out=colmin_out[:, o0 : o0 + MG],
                            in_=colmin[:, o0 : o0 + MG],
                        )

            nc.sync.dma_start(out=rowmin_out, in_=rowmin_parts[:])

    return nc


def _split_bf16(x):
    """Round-to-nearest bf16 hi/lo split: x ~= hi + lo to ~16 mantissa bits."""
    import ml_dtypes

    hi = x.astype(np.float32).astype(ml_dtypes.bfloat16)
    lo = (x.astype(np.float32) - hi.astype(np.float32)).astype(ml_dtypes.bfloat16)
    return hi, lo


def _aug16(pts, sq):
    """[16, n] bf16 lhsT-side augmentation rows for d2 via one K=16 matmul:
    [a_hi, a_lo, 1, 1, -2f_hi(3), -2f_lo(3), -2f_hi(3), -2f_lo(3)]."""
    import ml_dtypes

    bf = ml_dtypes.bfloat16
    n = pts.shape[0]
    a_hi, a_lo = _split_bf16(sq)
    f_hi, f_lo = _split_bf16(pts)
    f2_hi = (-2.0 * f_hi.astype(np.float32)).astype(bf)
    f2_lo = (-2.0 * f_lo.astype(np.float32)).astype(bf)
    ones = np.ones(n, bf)
    return np.stack(
        [a_hi, a_lo, ones, ones]
        + [f2_hi[:, d] for d in range(3)]
        + [f2_lo[:, d] for d in range(3)]
        + [f2_hi[:, d] for d in range(3)]
        + [f2_lo[:, d] for d in range(3)]
    )


def _aug16_rhs(pts, sq):
    """[16, m] bf16 rhs-side augmentation rows:
    [1, 1, b_hi, b_lo, p_hi(3), p_hi(3), p_lo(3), p_lo(3)]."""
    import ml_dtypes

    bf = ml_dtypes.bfloat16
    m = pts.shape[0]
    b_hi, b_lo = _split_bf16(sq)
    p_hi, p_lo = _split_bf16(pts)
    ones = np.ones(m, bf)
    return np.stack(
        [ones, ones, b_hi, b_lo]
        + [p_hi[:, d] for d in range(3)]
        + [p_hi[:, d] for d in range(3)]
        + [p_lo[:, d] for d in range(3)]
        + [p_lo[:, d] for d in range(3)]
    )


def _build_schedule(lengths):
    """Split every sample's valid n-tile range into 4-tile segments and pack
    them into 8 equal per-core lists (padded with dummy segments)."""
    segs = []  # (sample, first_tile)
    for s in range(B):
        ntiles = max(1, -(-int(lengths[s]) // P))  # ceil(L/128), >= 1
        for st in range(0, ntiles, TILES_PER_SEG):
            segs.append((s, st))
    seg_per_core = -(-len(segs) // B)
    while len(segs) < seg_per_core * B:
        segs.append(None)  # dummy
    cores = [segs[c * seg_per_core : (c + 1) * seg_per_core] for c in range(B)]
    return cores, seg_per_core


def _prep_inputs(fg, prj, lengths):
    import ml_dtypes

    bf = ml_dtypes.bfloat16
    cores, seg_per_core = _build_schedule(lengths)
    slots = seg_per_core * TILES_PER_SEG

    fg_f = fg.astype(np.float32)
    prj_f = prj.astype(np.float32)
    fg2 = (fg_f.astype(np.float64) ** 2).sum(-1).astype(np.float32)
    prj2 = (prj_f.astype(np.float64) ** 2).sum(-1).astype(np.float32)
    fg_aug = {s: _aug16(fg_f[s], fg2[s]) for s in range(B)}  # [16, N]
    prj_aug = {s: _aug16_rhs(prj_f[s], prj2[s]) for s in range(B)}  # [16, M]
    pad_cols = np.full((16, P * TILES_PER_SEG), 0, bf)
    pad_cols[0, :] = bf(BIG)  # d2 of dummy rows = BIG + prj2 - 0 >> any real d2

    in_maps = []
    for c in range(B):
        fg_c = np.empty((16, slots * P), bf)
        prj_c = np.empty((16, seg_per_core * M), bf)
        for k, seg in enumerate(cores[c]):
            lo = k * TILES_PER_SEG * P
            hi = lo + TILES_PER_SEG * P
            if seg is None:
                fg_c[:, lo:hi] = pad_cols
                prj_c[:, k * M : (k + 1) * M] = prj_aug[0]
            else:
                s, st = seg
                r0 = st * P
                fg_c[:, lo:hi] = fg_aug[s][:, r0 : r0 + TILES_PER_SEG * P]
                prj_c[:, k * M : (k + 1) * M] = prj_aug[s]
        in_maps.append(
            {"fg_c": np.ascontiguousarray(fg_c), "prj_c": np.ascontiguousarray(prj_c)}
        )
    return in_maps, cores, seg_per_core


def _combine(results, cores, seg_per_core, lengths):
    """Host-side reduction of the per-core partials to the scalar loss."""
    colmin = {}  # sample -> running [P, M] f32 min
    rowsum = np.zeros(B, np.float64)  # per-sample masked sum of rowmins
    for c in range(B):
        cm = np.asarray(results[c]["colmin_out"], dtype=np.float32)
        rm = np.asarray(results[c]["rowmin_out"], dtype=np.float32)
        cm = cm.reshape(P, seg_per_core, M)
        rm = rm.reshape(P, seg_per_core * TILES_PER_SEG)
        for k, seg in enumerate(cores[c]):
            if seg is None:
                continue
            s, st = seg
            prev = colmin.get(s)
            cur = cm[:, k, :]
            colmin[s] = cur if prev is None else np.minimum(prev, cur)
            L = int(lengths[s])
            for tt in range(TILES_PER_SEG):
                n0 = (st + tt) * P
                nvalid = min(max(L - n0, 0), P)
                if nvalid <= 0:
                    continue
                rmin = rm[:nvalid, k * TILES_PER_SEG + tt]
                rowsum[s] += np.maximum(rmin, 0.0).sum()
    total = 0.0
    for s in range(B):
        L = int(lengths[s])
        cham_x = rowsum[s] / L
        cham_y = np.maximum(colmin[s].min(axis=0), 0.0).mean()
        total += cham_x + cham_y
    return np.float32(total / B)


def _run(in_maps, seg_per_core, trace=False):
    from concourse.bass_utils import run_bass_kernel_spmd

    key = ("nc", seg_per_core)
    if key not in _cached:
        _cached[key] = _build_program(seg_per_core)
    return run_bass_kernel_spmd(_cached[key], in_maps, list(range(B)), trace=trace)


def kernel(fg_points, prj_points, x_lengths, _trace=False):
    fg = np.asarray(fg_points)
    prj = np.asarray(prj_points)
    lengths = np.asarray(x_lengths)
    in_maps, cores, seg_per_core = _prep_inputs(fg, prj, lengths)
    res = _run(in_maps, seg_per_core, trace=_trace)
    out = _combine(res.results, cores, seg_per_core, lengths)
    if _trace:
        return out, res
    return out
